# revision 18
# baseline (speedup 1.0000x reference)
"""Trainium2 Bass kernel for multi-head causal self-attention.

Reference computation (B=4, T=2048, E=1024, H=16 heads, D=64):
    qkv = x @ w_qkv;  q,k,v split
    scores = q @ k^T / sqrt(D),  causal + key-pad mask (input_ids==0)
    y = softmax(scores) @ v;  out = y @ w_out + b_out

Sharding over 8 cores: core c -> batch b = c//2, head-group g = c%2
(8 heads each).  Each core computes its heads' attention output and the
partial out-projection (contraction over its 512 y-dims); the host sums
the two partials per batch (w_out row-split tensor parallelism).

Per-core design (v3, cost-model-driven):
  - Projections as fp8e4m3 DoubleRow matmuls (0.5 cyc/col).  Host-side
    residual decomposition recovers accuracy: qk 2-term (X1W1+X1W2),
    v 3-term.  Weights pre-scaled by SC=32.
  - Scores fp8 DoubleRow with q,k requantized on the DVE during the
    psum->sbuf stage copy, pre-multiplied by sqrt(0.125)/SC so the
    softmax scale is baked in; an SBUF->SBUF DMA folds the 64 D-dims
    into [32 part, 2 k-tiles] DoubleRow layout plus ONE extra mask row
    per head (q-row = +240 const, k-row = -240 on padded keys (240 = TRN fp8e4 max)) so the
    pad mask lands in the matmul itself (exp arg -2e5 -> 0) and exp is
    a pure function (no per-block bias).
  - exp on Act, [128, span<=1024] per (head, J-half, key block).
  - PV transposed: p-chunk [128k,128q] as lhsT, v [128k,64] as rhs ->
    y accumulates in a packed [128q, 2, 64] psum bank (65 free cols of
    work per block-pair instead of 512): chain for query block qb runs
    i=0..qb sequentially right after exp(qb) (interleaved accumulation
    groups within a bank are broken on hw; sequential chains are fine).
    Denominator via a separate 1-col matmul against v's SC column into
    a [128,2] den bank.  Normalization = per-partition reciprocal +
    tensor_scalar_mul (queries live on partitions now).
  - y^T for the out-projection via DMA-engine xbar transpose
    (dma_start_transpose, [128,128] bf16 tiles): no PE/PSUM/DVE cost.
  - Unit order J-outer (all heads' queries [0:1024] first), so the
    out-projection for the first 8 t-blocks overlaps the second half's
    attention; the last unit streams transpose+out-proj per t-block.
  - Fill queue interleaves projection/out-proj matmuls into attention
    idle slots (Act is the bottleneck engine at ~152us).
"""

import numpy as np

B, T, E, H, D = 4, 2048, 1024, 16, 64
SC = 32.0          # weight pre-scale (keeps fp8 residuals out of subnormals)
ALPHA = float(np.sqrt(0.125) / SC)   # q/k requant prescale: alpha^2 = 1/(8*SC^2)
NKB = T // 128     # 16 key blocks
QK_TERMS = 2       # fp8 residual terms for the q/k projection (1, 2, or 3)
V_TERMS = 3        # fp8 residual terms for the v projection

_cache = {}


def _build_nc(qk_terms=QK_TERMS, v_terms=V_TERMS):
    import collections
    import concourse.mybir as mybir
    import concourse.tile as tile
    from concourse import bacc

    f32 = mybir.dt.float32
    bf16 = mybir.dt.bfloat16
    fp8 = mybir.dt.float8e4
    DR = mybir.MatmulPerfMode.DoubleRow
    Exp = mybir.ActivationFunctionType.Exp

    nc = bacc.Bacc("TRN2", target_bir_lowering=False)
    x8a_d = nc.dram_tensor("x8a", [128, 8, T], fp8, kind="ExternalInput")
    x8b_d = nc.dram_tensor("x8b", [128, 8, T], fp8, kind="ExternalInput")
    wqka_d = nc.dram_tensor("wqka", [4, 128, 8, 256], fp8, kind="ExternalInput")
    wqkb_d = nc.dram_tensor("wqkb", [4, 128, 8, 256], fp8, kind="ExternalInput")
    wva_d = nc.dram_tensor("wva", [128, 8, 512], fp8, kind="ExternalInput")
    wvb_d = nc.dram_tensor("wvb", [128, 8, 512], fp8, kind="ExternalInput")
    wout_d = nc.dram_tensor("wout", [4, 128, E], bf16, kind="ExternalInput")
    qrow_d = nc.dram_tensor("qrow", [2, 2, T], fp8, kind="ExternalInput")
    krow_d = nc.dram_tensor("krow", [2, 2, T], fp8, kind="ExternalInput")
    out_d = nc.dram_tensor("out", [T, E], f32, kind="ExternalOutput")

    with tile.TileContext(nc) as tc:
        with (
            tc.tile_pool(name="const", bufs=1) as cpool,
            tc.tile_pool(name="xp", bufs=1) as xpool,
            tc.tile_pool(name="wvp", bufs=1) as wvpool,
            tc.tile_pool(name="wqkp", bufs=3) as wqkpool,
            tc.tile_pool(name="wop", bufs=1) as wopool,
            tc.tile_pool(name="vp", bufs=1) as vpool,
            tc.tile_pool(name="stg", bufs=3) as stgpool,
            tc.tile_pool(name="qkf", bufs=4) as qkfpool,
            tc.tile_pool(name="ptp", bufs=20) as ptpool,
            tc.tile_pool(name="rp", bufs=4) as rpool,
            tc.tile_pool(name="ysbp", bufs=2) as ysbpool,
            tc.tile_pool(name="ytp", bufs=1) as ytpool,
            tc.tile_pool(name="otp", bufs=3) as otpool,
            tc.tile_pool(name="psA", bufs=2, space="PSUM") as psA,
            tc.tile_pool(name="psY", bufs=1, space="PSUM") as psY,
            tc.tile_pool(name="psD", bufs=1, space="PSUM") as psD,
            tc.tile_pool(name="psM", bufs=2, space="PSUM") as psM,
        ):
            # ---- constants & resident inputs ----
            qrow_sb = cpool.tile([2, 2, T], fp8, tag="qr", name="qrow_sb")
            krow_sb = cpool.tile([2, 2, T], fp8, tag="kr", name="krow_sb")
            nc.sync.dma_start(out=qrow_sb, in_=qrow_d[:, :, :])
            nc.sync.dma_start(out=krow_sb, in_=krow_d[:, :, :])

            x8a = xpool.tile([128, 8, T], fp8, tag="xa", name="x8a")
            x8b = xpool.tile([128, 8, T], fp8, tag="xb", name="x8b")
            wva = wvpool.tile([128, 8, 512], fp8, tag="wva", name="wva")
            wvb = wvpool.tile([128, 8, 512], fp8, tag="wvb", name="wvb")

            def x8_chunk(n):
                sl = slice(512 * n, 512 * n + 512)
                nc.sync.dma_start(out=x8a[:, :, sl], in_=x8a_d[:, :, sl])
                if v_terms >= 3:
                    nc.sync.dma_start(out=x8b[:, :, sl], in_=x8b_d[:, :, sl])

            wout_sb = []
            for s in range(4):
                t_ = wopool.tile([128, E], bf16, tag=f"wo{s}", name=f"wo{s}")
                wout_sb.append(t_)

            yT_all = [
                ytpool.tile([128, T], bf16, tag=f"yt{s}", name=f"yt{s}")
                for s in range(4)
            ]
            v_sb = [None] * NKB
            ysb_tiles = {}

            # x-side residual term list: (x_tile, w_is_resid) pairs
            def _terms(n):
                if n == 1:
                    return [(x8a, 0)]
                if n == 2:
                    return [(x8a, 0), (x8a, 1)]
                return [(x8a, 0), (x8a, 1), (x8b, 0)]

            # ---- emission helpers (closure lists, run in program order) ----
            def v_group_closures(tb):
                # one [128 t, 512 vcols] psum group: v' = SC * x @ w_v
                st = {}

                def init():
                    vt = vpool.tile([128, 8, 65], bf16, tag=f"v{tb}", name=f"v{tb}")
                    # col 64 survives the copy; value SC makes den = SC*sum(p)
                    # so 1/den absorbs the weight pre-scale for free
                    nc.gpsimd.memset(vt, SC)
                    st["vt"] = vt
                    st["vp"] = psM.tile([128, 512], f32, tag="proj", name="vps")
                    mms = []
                    for xt, wr in _terms(v_terms):
                        for g in range(4):
                            mms.append((xt, (wvb if wr else wva), g))
                    st["mms"] = mms

                def mm_i(mi):
                    xt, wt, g = st["mms"][mi]
                    nc.tensor.matmul(
                        st["vp"],
                        lhsT=xt[:, 2 * g : 2 * g + 2, 128 * tb : 128 * tb + 128],
                        rhs=wt[:, 2 * g : 2 * g + 2, :],
                        start=(mi == 0), stop=(mi == len(st["mms"]) - 1),
                        perf_mode=DR,
                    )

                def first():
                    init()
                    mm_i(0)

                cls = [first]
                for mi in range(1, 4 * v_terms):
                    cls.append(lambda mi=mi: mm_i(mi))

                def copy():
                    nc.vector.tensor_copy(
                        out=st["vt"][:, :, 0:64],
                        in_=st["vp"].rearrange("p (h c) -> p h c", c=64),
                    )
                    v_sb[tb] = st["vt"]

                cls.append(copy)
                return cls

            def emit_qk_pair(pair):
                """qk-projection for one head pair -> folded fp8 q8f/k8f with
                mask rows.  Returns closure list (run via the fill queue)."""
                state = {}

                def dma_w():
                    wa = wqkpool.tile([128, 8, 256], fp8, tag="wa", name="wqka_sb")
                    nc.sync.dma_start(out=wa, in_=wqka_d[pair])
                    state["wa"] = wa
                    if qk_terms > 1:
                        wb = wqkpool.tile([128, 8, 256], fp8, tag="wb", name="wqkb_sb")
                        nc.sync.dma_start(out=wb, in_=wqkb_d[pair])
                        state["wb"] = wb
                    state["sq"] = stgpool.tile([128, T], fp8, tag="sq", name="stage_q")
                    state["sk"] = stgpool.tile([128, T], fp8, tag="sk", name="stage_k")

                closures = [dma_w]

                def mk_group(chunk, n):
                    # psum group: 128 qk-cols (chunk 0=q, 1=k), T-slice n
                    def go():
                        qp = psM.tile([128, 512], f32, tag="proj", name="qkps")
                        state["qp"] = qp
                        mms = []
                        for xt, wr in _terms(qk_terms):
                            for g in range(4):
                                mms.append((xt, state["wb"] if wr else state["wa"], g))
                        state["mms"] = mms

                        def mm_i(mi):
                            xt, wt, g = state["mms"][mi]
                            nc.tensor.matmul(
                                state["qp"],
                                lhsT=wt[:, 2 * g : 2 * g + 2,
                                        128 * chunk : 128 * chunk + 128],
                                rhs=xt[:, 2 * g : 2 * g + 2, 512 * n : 512 * n + 512],
                                start=(mi == 0), stop=(mi == len(state["mms"]) - 1),
                                perf_mode=DR,
                            )
                        return mm_i
                    def first():
                        state["mm_i"] = go()
                        state["mm_i"](0)
                    res = [first]
                    for mi in range(1, 4 * qk_terms):
                        res.append(lambda mi=mi: state["mm_i"](mi))
                    def copy():
                        # requant to fp8 with the softmax scale baked in
                        stage = state["sq"] if chunk == 0 else state["sk"]
                        nc.vector.tensor_scalar_mul(
                            stage[:, 512 * n : 512 * n + 512], state["qp"], ALPHA
                        )
                    res.append(copy)
                    return res

                def alloc_folds():
                    # [97, 2, T]: head h rows [64h:64h+32] = q/k dims,
                    # row 64h+32 = mask row (PE operand base must be 0/64)
                    state["qf"] = qkfpool.tile([128, 2, T], fp8, tag="qf", name="q8f")
                    state["kf"] = qkfpool.tile([128, 2, T], fp8, tag="kf", name="k8f")
                    nc.sync.dma_start(out=state["qf"][32::64, :, :], in_=qrow_sb)
                    nc.sync.dma_start(out=state["kf"][32::64, :, :], in_=krow_sb)

                def folds(thalf):
                    tsl = slice(1024 * thalf, 1024 * thalf + 1024)
                    for dst, src in ((state["qf"], state["sq"]), (state["kf"], state["sk"])):
                        for h in range(2):
                            for j in range(2):
                                nc.sync.dma_start(
                                    out=dst[64 * h : 64 * h + 32, j, tsl],
                                    in_=src[64 * h + 32 * j : 64 * h + 32 * j + 32, tsl],
                                )

                closures.append(alloc_folds)
                for n in (0, 1):
                    for chunk in range(2):
                        closures.extend(mk_group(chunk, n))
                closures.append(lambda: folds(0))
                closures.append(f"qk{pair}h0")
                for n in (2, 3):
                    for chunk in range(2):
                        closures.extend(mk_group(chunk, n))
                closures.append(lambda: folds(1))
                closures.append(f"qk{pair}h1")
                return closures, state

            def emit_out_tb(tb):
                ot = otpool.tile([128, E], f32, tag="ot", name="ot")
                st = {}
                cls = []
                for n2 in range(2):
                    def first(n2=n2):
                        op = psM.tile([128, 512], f32, tag="proj", name="ops")
                        st["op"] = op
                        nc.tensor.matmul(
                            op,
                            lhsT=yT_all[0][:, 128 * tb : 128 * tb + 128],
                            rhs=wout_sb[0][:, 512 * n2 : 512 * n2 + 512],
                            start=True, stop=False,
                        )
                    cls.append(first)
                    for s in range(1, 4):
                        def mid(s=s, n2=n2):
                            nc.tensor.matmul(
                                st["op"],
                                lhsT=yT_all[s][:, 128 * tb : 128 * tb + 128],
                                rhs=wout_sb[s][:, 512 * n2 : 512 * n2 + 512],
                                start=False, stop=(s == 3),
                            )
                        cls.append(mid)
                    def copy(n2=n2):
                        nc.vector.tensor_copy(
                            out=ot[:, 512 * n2 : 512 * n2 + 512], in_=st["op"]
                        )
                    cls.append(copy)
                def dma():
                    nc.sync.dma_start(
                        out=out_d[128 * tb : 128 * tb + 128, :], in_=ot
                    )
                cls.append(dma)
                return cls

            # ---- fill queue machinery ----
            fill_q = collections.deque()
            markers_seen = set()

            def fill(n):
                done = 0
                while done < n and fill_q:
                    item = fill_q.popleft()
                    if isinstance(item, str):
                        markers_seen.add(item)
                        continue
                    item()
                    done += 1

            def drain_to(marker):
                while marker not in markers_seen:
                    item = fill_q.popleft()
                    if isinstance(item, str):
                        markers_seen.add(item)
                        continue
                    item()

            # ---- attention unit: one (head, J query-half) ----
            def emit_unit(pair_state, J, h, lh, pair, pull, post_norm=None):
                qf, kf = pair_state["qf"], pair_state["kf"]
                kT = kf[64 * h : 64 * h + 33]
                qT = qf[64 * h : 64 * h + 33]
                nblk = 8 * (J + 1)
                y_ps = psY.tile([128, 2, 64], f32, tag="y", name="y_ps")
                den_ps = psD.tile([128, 2], f32, tag="den", name="den_ps")
                pts = [None] * nblk

                def chain(qb):
                    # full PV chain + normalization for global query block qb
                    qb_l = qb - 8 * J
                    slot = qb_l % 2
                    if v_sb[qb] is None:
                        drain_to(f"v{qb}")
                    for i2 in range(qb + 1):
                        lhsT = pts[i2][:, 128 * qb_l : 128 * qb_l + 128]
                        nc.tensor.matmul(
                            den_ps[:, slot : slot + 1],
                            lhsT=lhsT,
                            rhs=v_sb[i2][:, lh, 64:65],
                            start=(i2 == 0), stop=(i2 == qb),
                        )
                        nc.tensor.matmul(
                            y_ps[:, slot, :],
                            lhsT=lhsT,
                            rhs=v_sb[i2][:, lh, 0:64],
                            start=(i2 == 0), stop=(i2 == qb),
                        )
                    rt = rpool.tile([128, 1], f32, tag="rt", name="rt")
                    nc.vector.reciprocal(out=rt, in_=den_ps[:, slot : slot + 1])
                    key = (pair, qb)
                    if key not in ysb_tiles:
                        ysb_tiles[key] = ysbpool.tile(
                            [128, 2, 64], bf16, tag=f"ysb{qb}", name=f"ysb{qb}"
                        )
                    nc.vector.tensor_scalar_mul(
                        ysb_tiles[key][:, h, :], y_ps[:, slot, :], rt
                    )
                    if post_norm is not None:
                        post_norm(qb)

                s2s = [None] * nblk

                def scores(i):
                    c0 = max(0, 128 * i - 1024 * J)
                    s2 = psA.tile([128, 1024], f32, tag="s", name="s2")
                    s2s[i] = s2
                    kblk = kT[:, :, 128 * i : 128 * i + 128]
                    if c0 < 512:
                        nc.tensor.matmul(
                            s2[:, c0:512],
                            lhsT=kblk,
                            rhs=qT[:, :, 1024 * J + c0 : 1024 * J + 512],
                            start=True, stop=True, perf_mode=DR,
                        )
                        nc.tensor.matmul(
                            s2[:, 512:1024],
                            lhsT=kblk,
                            rhs=qT[:, :, 1024 * J + 512 : 1024 * J + 1024],
                            start=True, stop=True, perf_mode=DR,
                        )
                    else:
                        nc.tensor.matmul(
                            s2[:, c0:1024],
                            lhsT=kblk,
                            rhs=qT[:, :, 1024 * J + c0 : 1024 * J + 1024],
                            start=True, stop=True, perf_mode=DR,
                        )

                scores(0)
                prev = None
                for i in range(nblk):
                    c0 = max(0, 128 * i - 1024 * J)
                    pt_t = ptpool.tile([128, 1024], bf16, tag="pt", name="pt")
                    pts[i] = pt_t
                    nc.scalar.activation(
                        out=pt_t[:, c0:1024],
                        in_=s2s[i][:, c0:1024],
                        func=Exp,
                    )
                    if 128 * i >= 1024 * J:
                        nc.gpsimd.affine_select(
                            out=pt_t[:, c0 : c0 + 128],
                            in_=pt_t[:, c0 : c0 + 128],
                            compare_op=mybir.AluOpType.is_ge,
                            fill=0.0, base=0,
                            pattern=[[1, 128]], channel_multiplier=-1,
                        )
                    # next block's scores jump ahead of the chain matmuls so
                    # the Act engine never waits behind them
                    if i + 1 < nblk:
                        scores(i + 1)
                    fill(pull)
                    if prev is not None and prev >= 8 * J:
                        chain(prev)
                    prev = i
                chain(prev)

            # ==== orchestration ====
            # prologue (direct): first half of pair-0's qk projection (enough
            # for J=0) and v for key blocks 0-7.
            qk0_closures, pstate0 = emit_qk_pair(0)
            # [dma_w, alloc_folds] + 4 groups + fold0 (+marker)
            n_pro = 2 + 4 * (4 * qk_terms + 1) + 1
            qk0_closures[0]()          # wqk DMAs + stage alloc
            x8_chunk(0)
            nc.sync.dma_start(out=wva, in_=wva_d[:, :, :])
            if v_terms >= 2:
                nc.sync.dma_start(out=wvb, in_=wvb_d[:, :, :])
            x8_chunk(1)
            for c in qk0_closures[1:n_pro]:
                if isinstance(c, str):
                    markers_seen.add(c)
                else:
                    c()
            markers_seen.add("qk0h0")
            for tb in range(8):
                for c in v_group_closures(tb):
                    c()
                markers_seen.add(f"v{tb}")

            # fill queue: x8 tail, rest of qk0, v 8-15, wout, qk1
            # (qk2/qk3 are queued when pairs 1/2 start, as in the baseline,
            # so drains never force large not-yet-ready bursts)
            fill_q.append(lambda: x8_chunk(2))
            fill_q.append(lambda: x8_chunk(3))
            fill_q.extend(qk0_closures[n_pro:])
            for tb in range(8, 12):
                fill_q.extend(v_group_closures(tb))
                fill_q.append(f"v{tb}")
            for s in range(4):
                fill_q.append(lambda s=s: nc.sync.dma_start(out=wout_sb[s], in_=wout_d[s]))
            pair_states = [pstate0, None, None, None]
            cl1, st1 = emit_qk_pair(1)
            pair_states[1] = st1
            # qk1's weight DMA + first half, v-groups interleaved so PE has
            # ready work while pair-1 weights/stages are in flight
            h0_idx = cl1.index("qk1h0")
            fill_q.extend(cl1[: 2])          # dma_w + alloc_folds (DMAs early)
            for tb in range(12, 16):
                fill_q.extend(v_group_closures(tb))
                fill_q.append(f"v{tb}")
            fill_q.extend(cl1[2 : h0_idx + 1])
            fill_q.extend(cl1[h0_idx + 1 :])

            def transpose_tb(pair, tb):
                nc.sync.dma_start_transpose(
                    out=yT_all[pair][:, 128 * tb : 128 * tb + 128],
                    in_=ysb_tiles[(pair, tb)].rearrange("p a b -> p (a b)"),
                )

            # pair-outer unit order (spreads projection load across the
            # whole timeline; J1 units are Act-heavy and hide fill work)
            def run_phase(pair, J):
                drain_to(f"qk{pair}h{J}")
                for h in range(2):
                    lh = 2 * pair + h
                    pull = 2 if J == 0 else 3
                    last = pair == 3 and J == 1 and h == 1
                    post_norm = None
                    if last:
                        # stream transpose + out-proj per tb in the tail
                        def post_norm(qb):
                            transpose_tb(pair, qb)
                            for c in emit_out_tb(qb):
                                c()
                    elif h == 1:
                        def post_norm(qb):
                            transpose_tb(pair, qb)
                    emit_unit(pair_states[pair], J, h, lh, pair, pull,
                              post_norm)
                if pair == 3 and J == 0:
                    # all pairs' first-half yT ready: out-proj tb 0-7
                    # fills pair 3's J1 attention
                    for tb in range(8):
                        fill_q.extend(emit_out_tb(tb))

            def queue_qk(p):
                cl, st = emit_qk_pair(p)
                pair_states[p] = st
                fill_q.extend(cl)

            # phase order pulls J0 halves forward so the first-half
            # out-projection overlaps the last two J1 phases
            run_phase(0, 0)
            queue_qk(2)
            run_phase(0, 1)
            run_phase(1, 0)
            queue_qk(3)
            run_phase(2, 0)
            run_phase(1, 1)
            run_phase(3, 0)
            run_phase(2, 1)
            run_phase(3, 1)
            # drain leftovers
            while fill_q:
                item = fill_q.popleft()
                if not isinstance(item, str):
                    item()

    nc.compile()
    return nc


def _q8(a):
    import ml_dtypes
    return np.ascontiguousarray(a.astype(ml_dtypes.float8_e4m3))


def _fold(a):
    """[E, C] -> [128, 8, C] with (p, 2g+j) = row 256g + 128j + p."""
    Edim, C = a.shape
    return np.ascontiguousarray(
        a.reshape(4, 2, 128, C).transpose(2, 0, 1, 3).reshape(128, 8, C)
    )


def _prep_in_maps(x, input_ids, w_qkv, w_out):
    import ml_dtypes

    in_maps = []
    for c in range(8):
        b, g = divmod(c, 2)
        hbase = 8 * g
        xT = np.ascontiguousarray(np.asarray(x[b], dtype=np.float32).T)  # [E, T]
        x8a = xT.astype(ml_dtypes.float8_e4m3).astype(np.float32)
        x8b = xT - x8a
        # q/k cols for the 4 pairs: [qA qB kA kB] each 64 -> [E, 256] per pair
        wqka = np.empty((4, 128, 8, 256), ml_dtypes.float8_e4m3)
        wqkb = np.empty((4, 128, 8, 256), ml_dtypes.float8_e4m3)
        for p in range(4):
            ha = hbase + 2 * p
            wc = np.concatenate(
                [
                    w_qkv[:, 64 * ha : 64 * ha + 128],
                    w_qkv[:, E + 64 * ha : E + 64 * ha + 128],
                ],
                axis=1,
            ) * SC
            wa = wc.astype(ml_dtypes.float8_e4m3).astype(np.float32)
            wqka[p] = _fold(wa).astype(ml_dtypes.float8_e4m3)
            wqkb[p] = _fold(wc - wa).astype(ml_dtypes.float8_e4m3)
        wv = np.asarray(
            w_qkv[:, 2 * E + 64 * hbase : 2 * E + 64 * (hbase + 8)], np.float32
        ) * SC
        wva = wv.astype(ml_dtypes.float8_e4m3).astype(np.float32)
        wvaf = _fold(wva).astype(ml_dtypes.float8_e4m3)
        wvbf = _fold(wv - wva).astype(ml_dtypes.float8_e4m3)
        wout = np.ascontiguousarray(
            np.asarray(w_out[512 * g : 512 * g + 512, :], np.float32).reshape(
                4, 128, E
            ).astype(ml_dtypes.bfloat16)
        )
        qrow = np.zeros((2, 2, T), np.float32)
        qrow[:, 0, :] = 240.0
        krow = np.zeros((2, 2, T), np.float32)
        krow[:, 0, :] = np.where(np.asarray(input_ids[b]) != 0, 0.0, -240.0)
        in_maps.append({
            "x8a": _q8(_fold(x8a)),
            "x8b": _q8(_fold(x8b)),
            "wqka": np.ascontiguousarray(wqka),
            "wqkb": np.ascontiguousarray(wqkb),
            "wva": wvaf,
            "wvb": wvbf,
            "wout": wout,
            "qrow": _q8(qrow),
            "krow": _q8(krow),
        })
    return in_maps


def kernel(x, input_ids, w_qkv, w_out, b_out, _trace=False):
    from concourse import bass_utils

    x = np.asarray(x, dtype=np.float32)
    w_qkv = np.asarray(w_qkv, dtype=np.float32)
    w_out = np.asarray(w_out, dtype=np.float32)
    b_out = np.asarray(b_out, dtype=np.float32)

    key = (QK_TERMS, V_TERMS)
    if key not in _cache:
        _cache[key] = _build_nc(QK_TERMS, V_TERMS)
    nc = _cache[key]

    in_maps = _prep_in_maps(x, input_ids, w_qkv, w_out)
    res = bass_utils.run_bass_kernel_spmd(
        nc, in_maps, core_ids=list(range(8)), trace=_trace,
    )
    out = np.empty((B, T, E), np.float32)
    for b in range(B):
        out[b] = res.results[2 * b]["out"] + res.results[2 * b + 1]["out"] + b_out
    if _trace:
        kernel.last_result = res
    return out


# revision 19
# speedup vs baseline: 1.2851x; 1.2851x over previous
"""Trainium2 Bass kernel for multi-head causal self-attention.

Reference computation (B=4, T=2048, E=1024, H=16 heads, D=64):
    qkv = x @ w_qkv;  q,k,v split
    scores = q @ k^T / sqrt(D),  causal + key-pad mask (input_ids==0)
    y = softmax(scores) @ v;  out = y @ w_out + b_out

Sharding over 8 cores: core c -> batch b = c//2, head-group g = c%2
(8 heads each).  Each core computes its heads' attention output and the
partial out-projection (contraction over its 512 y-dims); the host sums
the two partials per batch (w_out row-split tensor parallelism).

Per-core kernel strategy (cost-model-driven):
  - All projections run as fp8e4m3 DoubleRow matmuls (0.5 cycles/row,
    256-deep contraction per instruction = 4x fp32r throughput).
    Host-side residual decomposition (x = X1+X2, w = W1+W2 at fp8,
    computing X1W1+X1W2+X2W1) recovers ~bf16 accuracy.  Weights are
    pre-scaled by SC=32 so the residual terms stay out of fp8's
    subnormal range; the inverse scale is folded into the exp scale
    (scores) and the softmax-normalization constant (v path) for free.
  - Scores also run as fp8 DoubleRow: q,k are quantized to fp8 on the
    DVE during the psum->sbuf copy, and an SBUF->SBUF DMA folds the
    64 D-dims into [32 partitions, 2 k-tiles] DoubleRow layout.
  - Scores are computed transposed, sT [keys, q], so softmax(p) @ v is
    a matmul contracting over keys with v in natural [T, D] layout; a
    ones-column appended to v yields the denominator for free.
  - Exp (scalar engine) processes a [128, 1024] 2-psum-bank span per
    key block (both query halves in one instruction; the key-pad mask
    is a per-partition bias so one call per key block is legal).
    Softmax skips max-subtraction (scores are O(3); exp arg bounded).
  - Causal masking: off-diagonal blocks skipped; diagonal 128x128
    blocks exp'd unmasked, then the lower triangle of p is zeroed with
    a gpsimd affine_select (keeps DVE/Act off the critical path).
  - Software pipelining: QKV-projection matmuls for the next head pair
    (and the deferred V/out-projection work) are interleaved into the
    attention block loop through a fill queue, so the PE stays busy
    during the Act-bound attention phase.
"""

import numpy as np

B, T, E, H, D = 4, 2048, 1024, 16, 64
NEG = -1e30
SC = 32.0          # weight pre-scale (keeps fp8 residuals out of subnormals)
NKB = T // 128     # 16 key blocks
QK_TERMS = 2       # fp8 residual terms for the q/k projection (1, 2, or 3)
V_TERMS = 3        # fp8 residual terms for the v projection

_cache = {}


def _build_nc(qk_terms=QK_TERMS, v_terms=V_TERMS):
    import collections
    import concourse.mybir as mybir
    import concourse.tile as tile
    from concourse import bacc

    f32 = mybir.dt.float32
    bf16 = mybir.dt.bfloat16
    fp8 = mybir.dt.float8e4
    DR = mybir.MatmulPerfMode.DoubleRow
    Exp = mybir.ActivationFunctionType.Exp
    EXP_SCALE = 0.125 / (SC * SC)

    nc = bacc.Bacc("TRN2", target_bir_lowering=False)
    x8a_d = nc.dram_tensor("x8a", [128, 8, T], fp8, kind="ExternalInput")
    x8b_d = nc.dram_tensor("x8b", [128, 8, T], fp8, kind="ExternalInput")
    wqka_d = nc.dram_tensor("wqka", [4, 128, 8, 256], fp8, kind="ExternalInput")
    wqkb_d = nc.dram_tensor("wqkb", [4, 128, 8, 256], fp8, kind="ExternalInput")
    wva_d = nc.dram_tensor("wva", [128, 8, 512], fp8, kind="ExternalInput")
    wvb_d = nc.dram_tensor("wvb", [128, 8, 512], fp8, kind="ExternalInput")
    wout_d = nc.dram_tensor("wout", [4, 128, E], bf16, kind="ExternalInput")
    km_d = nc.dram_tensor("km", [128, NKB], f32, kind="ExternalInput")
    out_d = nc.dram_tensor("out", [T, E], f32, kind="ExternalOutput")

    with tile.TileContext(nc) as tc:
        with (
            tc.tile_pool(name="const", bufs=1) as cpool,
            tc.tile_pool(name="xp", bufs=1) as xpool,
            tc.tile_pool(name="wvp", bufs=1) as wvpool,
            tc.tile_pool(name="wqkp", bufs=2) as wqkpool,
            tc.tile_pool(name="wop", bufs=1) as wopool,
            tc.tile_pool(name="vp", bufs=1) as vpool,
            tc.tile_pool(name="stg", bufs=2) as stgpool,
            tc.tile_pool(name="qkf", bufs=2) as qkfpool,
            tc.tile_pool(name="ptp", bufs=6) as ptpool,
            tc.tile_pool(name="rp", bufs=2) as rpool,
            tc.tile_pool(name="ysbp", bufs=2) as ysbpool,
            tc.tile_pool(name="ytp", bufs=1) as ytpool,
            tc.tile_pool(name="otp", bufs=3) as otpool,
            tc.tile_pool(name="psA", bufs=2, space="PSUM") as psA,
            tc.tile_pool(name="psY", bufs=2, space="PSUM") as psY,
            tc.tile_pool(name="psM", bufs=2, space="PSUM") as psM,
        ):
            # ---- constants & resident inputs ----
            km_sb = cpool.tile([128, NKB], f32, tag="km", name="km_sb")
            nc.sync.dma_start(out=km_sb, in_=km_d[:, :])
            ones_inv = cpool.tile([65, 64], bf16, tag="oi", name="ones_inv")
            nc.vector.memset(ones_inv, 1.0 / SC)

            # x8 DMA'd in T-chunks; order: chunk0+wv first (v-proj needs them),
            # chunks 1-3 after, wout (needed only at out-proj) last.
            x8a = xpool.tile([128, 8, T], fp8, tag="xa", name="x8a")
            x8b = xpool.tile([128, 8, T], fp8, tag="xb", name="x8b")
            wva = wvpool.tile([128, 8, 512], fp8, tag="wva", name="wva")
            wvb = wvpool.tile([128, 8, 512], fp8, tag="wvb", name="wvb")

            def x8_chunk(n):
                sl = slice(512 * n, 512 * n + 512)
                nc.sync.dma_start(out=x8a[:, :, sl], in_=x8a_d[:, :, sl])
                nc.sync.dma_start(out=x8b[:, :, sl], in_=x8b_d[:, :, sl])

            wout_sb = []
            for s in range(4):
                t_ = wopool.tile([128, E], bf16, tag=f"wo{s}", name=f"wo{s}")
                wout_sb.append(t_)

            yT_all = [
                ytpool.tile([128, T], bf16, tag=f"yt{s}", name=f"yt{s}")
                for s in range(4)
            ]
            v_sb = [None] * NKB

            # x-side residual term list for a projection with given #terms:
            # (x_tile, w_is_resid) pairs
            def _terms(n):
                if n == 1:
                    return [(x8a, 0)]
                if n == 2:
                    return [(x8a, 0), (x8a, 1)]
                return [(x8a, 0), (x8a, 1), (x8b, 0)]

            # ---- emission helpers (closure lists, executed in program order) ----
            def v_group_closures(tb):
                # one [128 t, 512 vcols] psum group: v' = SC * x @ w_v
                st = {}

                def init():
                    vt = vpool.tile([128, 8, 65], bf16, tag=f"v{tb}", name=f"v{tb}")
                    # col 64 survives the copy; value SC makes den = SC*sum(p)
                    # so 1/den absorbs the weight pre-scale for free
                    nc.gpsimd.memset(vt, SC)
                    st["vt"] = vt
                    st["vp"] = psM.tile([128, 512], f32, tag="proj", name="vps")
                    mms = []
                    for xt, wr in _terms(v_terms):
                        for g in range(4):
                            mms.append((xt, (wvb if wr else wva), g))
                    st["mms"] = mms

                def mm_i(mi):
                    xt, wt, g = st["mms"][mi]
                    nc.tensor.matmul(
                        st["vp"],
                        lhsT=xt[:, 2 * g : 2 * g + 2, 128 * tb : 128 * tb + 128],
                        rhs=wt[:, 2 * g : 2 * g + 2, :],
                        start=(mi == 0), stop=(mi == len(st["mms"]) - 1),
                        perf_mode=DR,
                    )

                def first():
                    init()
                    mm_i(0)

                cls = [first]
                for mi in range(1, 4 * v_terms):
                    cls.append(lambda mi=mi: mm_i(mi))

                def copy():
                    nc.vector.tensor_copy(
                        out=st["vt"][:, :, 0:64],
                        in_=st["vp"].rearrange("p (h c) -> p h c", c=64),
                    )
                    v_sb[tb] = st["vt"]

                cls.append(copy)
                return cls

            def emit_qk_pair(pair):
                """qk-projection for one head pair -> folded fp8 q8f/k8f.
                Returns list of closures (executed via the fill queue)."""
                state = {}

                def dma_w():
                    wa = wqkpool.tile([128, 8, 256], fp8, tag="wa", name="wqka_sb")
                    nc.sync.dma_start(out=wa, in_=wqka_d[pair])
                    state["wa"] = wa
                    if qk_terms > 1:
                        wb = wqkpool.tile([128, 8, 256], fp8, tag="wb", name="wqkb_sb")
                        nc.sync.dma_start(out=wb, in_=wqkb_d[pair])
                        state["wb"] = wb
                    state["sq"] = stgpool.tile([128, T], fp8, tag="sq", name="stage_q")
                    state["sk"] = stgpool.tile([128, T], fp8, tag="sk", name="stage_k")

                closures = [dma_w]

                def mk_group(chunk, n):
                    # psum group: 128 qk-cols (chunk 0=q, 1=k), T-slice n
                    def go():
                        qp = psM.tile([128, 512], f32, tag="proj", name="qkps")
                        state["qp"] = qp
                        mms = []
                        for xt, wr in _terms(qk_terms):
                            for g in range(4):
                                mms.append((xt, state["wb"] if wr else state["wa"], g))
                        state["mms"] = mms

                        def mm_i(mi):
                            xt, wt, g = state["mms"][mi]
                            nc.tensor.matmul(
                                state["qp"],
                                lhsT=wt[:, 2 * g : 2 * g + 2,
                                        128 * chunk : 128 * chunk + 128],
                                rhs=xt[:, 2 * g : 2 * g + 2, 512 * n : 512 * n + 512],
                                start=(mi == 0), stop=(mi == len(state["mms"]) - 1),
                                perf_mode=DR,
                            )
                        return mm_i
                    # one closure creating the group+first mm, then the rest
                    def first():
                        state["mm_i"] = go()
                        state["mm_i"](0)
                    res = [first]
                    for mi in range(1, 4 * qk_terms):
                        res.append(lambda mi=mi: state["mm_i"](mi))
                    def copy():
                        stage = state["sq"] if chunk == 0 else state["sk"]
                        nc.vector.tensor_copy(
                            out=stage[:, 512 * n : 512 * n + 512], in_=state["qp"]
                        )
                    res.append(copy)
                    return res

                def alloc_folds():
                    state["qf"] = qkfpool.tile([64, 2, T], fp8, tag="qf", name="q8f")
                    state["kf"] = qkfpool.tile([64, 2, T], fp8, tag="kf", name="k8f")

                def folds(thalf):
                    tsl = slice(1024 * thalf, 1024 * thalf + 1024)
                    for dst, src in ((state["qf"], state["sq"]), (state["kf"], state["sk"])):
                        for h in range(2):
                            for j in range(2):
                                nc.sync.dma_start(
                                    out=dst[32 * h : 32 * h + 32, j, tsl],
                                    in_=src[64 * h + 32 * j : 64 * h + 32 * j + 32, tsl],
                                )

                closures.append(alloc_folds)
                # T-slices n=0,1 for both chunks first, then fold half 0,
                # then n=2,3 and fold half 1
                for n in (0, 1):
                    for chunk in range(2):
                        closures.extend(mk_group(chunk, n))
                closures.append(lambda: folds(0))
                for n in (2, 3):
                    for chunk in range(2):
                        closures.extend(mk_group(chunk, n))
                closures.append(lambda: folds(1))
                return closures, state

            def emit_out_tb(tb):
                ot = otpool.tile([128, E], f32, tag="ot", name="ot")
                cls = []
                for n2 in range(2):
                    def first(n2=n2):
                        op = psM.tile([128, 512], f32, tag="proj", name="ops")
                        emit_out_tb.op = op
                        nc.tensor.matmul(
                            op,
                            lhsT=yT_all[0][:, 128 * tb : 128 * tb + 128],
                            rhs=wout_sb[0][:, 512 * n2 : 512 * n2 + 512],
                            start=True, stop=False,
                        )
                    cls.append(first)
                    for s in range(1, 4):
                        def mid(s=s, n2=n2):
                            nc.tensor.matmul(
                                emit_out_tb.op,
                                lhsT=yT_all[s][:, 128 * tb : 128 * tb + 128],
                                rhs=wout_sb[s][:, 512 * n2 : 512 * n2 + 512],
                                start=False, stop=(s == 3),
                            )
                        cls.append(mid)
                    def copy(n2=n2):
                        nc.vector.tensor_copy(
                            out=ot[:, 512 * n2 : 512 * n2 + 512], in_=emit_out_tb.op
                        )
                    cls.append(copy)
                def dma():
                    nc.sync.dma_start(
                        out=out_d[128 * tb : 128 * tb + 128, :], in_=ot
                    )
                cls.append(dma)
                return cls

            # ---- fill queue machinery ----
            fill_q = collections.deque()
            markers_seen = set()

            def fill(n):
                done = 0
                while done < n and fill_q:
                    item = fill_q.popleft()
                    if isinstance(item, str):
                        markers_seen.add(item)
                        continue
                    item()
                    done += 1

            def drain_to(marker):
                while marker not in markers_seen:
                    item = fill_q.popleft()
                    if isinstance(item, str):
                        markers_seen.add(item)
                        continue
                    item()

            # ---- attention unit ----
            gpend = []             # deferred norm closures, cross-unit

            def emit_unit(pair_state, J, h, lh, pull, post_cb=None):
                qf, kf = pair_state["qf"], pair_state["kf"]
                kT = kf[32 * h : 32 * h + 32]
                qT = qf[32 * h : 32 * h + 32]
                nblk = 8 * J + 8
                y_half = [None, None]
                pend = gpend       # deferred closures (norm part2)
                prev = None        # (i, c0, pt_tile)

                def norms_for(i, c0, pt_t):
                    # emit pv for block, plus norm scheduling
                    for q2 in (0, 1):
                        lo = 512 * q2
                        ql = max(c0, lo)
                        if ql >= lo + 512:
                            continue
                        nlast = 8 * J + 3 if q2 == 0 else nblk - 1
                        if i == 0:
                            y_half[q2] = psY.tile([65, 512], f32, tag="y", name="yh")
                        nc.tensor.matmul(
                            y_half[q2][:, ql - lo : 512],
                            lhsT=v_sb[i][:, lh, :],
                            rhs=pt_t[:, ql : lo + 512],
                            start=(i == 0), stop=(i == nlast),
                        )
                        if i == nlast:
                            yh = y_half[q2]
                            # r = 1/(SC*den) at partition 0, then gpsimd
                            # broadcast to 64 partitions (SBUF, so the final
                            # mul has only one PSUM input)
                            rt = rpool.tile([65, 512], bf16, tag="r", name="rt")
                            with nc.allow_low_precision(reason="1/den in bf16; y is bf16 anyway"):
                                nc.vector.reciprocal(
                                    out=rt[0:1, :], in_=yh[64:65, :]
                                )
                            bc_sb = ysbpool.tile([64, 512], bf16, tag="ysb", name="bc_sb")
                            nc.gpsimd.partition_broadcast(bc_sb, rt[0:1, :])
                            def part2(q2=q2, yh=yh, bc_sb=bc_sb):
                                nc.vector.tensor_mul(
                                    yT_all[lh // 2][
                                        64 * (lh % 2) : 64 * (lh % 2) + 64,
                                        1024 * J + 512 * q2 : 1024 * J + 512 * q2 + 512,
                                    ],
                                    yh[0:64, :],
                                    bc_sb,
                                )
                            pend.append(part2)

                for i in range(nblk):
                    c0 = max(0, 128 * i - 1024 * J)
                    s2 = psA.tile([128, 1024], f32, tag="s", name="s2")
                    kblk = kT[:, :, 128 * i : 128 * i + 128]
                    if c0 < 512:
                        nc.tensor.matmul(
                            s2[:, c0:512],
                            lhsT=kblk,
                            rhs=qT[:, :, 1024 * J + c0 : 1024 * J + 512],
                            start=True, stop=True, perf_mode=DR,
                        )
                        nc.tensor.matmul(
                            s2[:, 512:1024],
                            lhsT=kblk,
                            rhs=qT[:, :, 1024 * J + 512 : 1024 * J + 1024],
                            start=True, stop=True, perf_mode=DR,
                        )
                    else:
                        nc.tensor.matmul(
                            s2[:, c0:1024],
                            lhsT=kblk,
                            rhs=qT[:, :, 1024 * J + c0 : 1024 * J + 1024],
                            start=True, stop=True, perf_mode=DR,
                        )
                    pt_t = ptpool.tile([128, 1024], bf16, tag="pt", name="pt")
                    nc.scalar.activation(
                        out=pt_t[:, c0:1024],
                        in_=s2[:, c0:1024],
                        func=Exp,
                        bias=km_sb[:, i : i + 1],
                        scale=EXP_SCALE,
                    )
                    if 128 * i >= 1024 * J:
                        nc.gpsimd.affine_select(
                            out=pt_t[:, c0 : c0 + 128],
                            in_=pt_t[:, c0 : c0 + 128],
                            compare_op=mybir.AluOpType.is_ge,
                            fill=0.0, base=0,
                            pattern=[[1, 128]], channel_multiplier=-1,
                        )
                    while pend:
                        pend.pop(0)()
                    if post_cb is not None and i in post_cb:
                        post_cb[i]()
                    fill(pull)
                    if prev is not None:
                        # hard ordering guarantee: v tile for block `prev`
                        # must be emitted before its pv matmul
                        if v_sb[prev[0]] is None:
                            drain_to(f"v{prev[0]}")
                        norms_for(*prev)
                    prev = (i, c0, pt_t)
                # final block's pv+norms; norm part2 closures stay in gpend
                # and are emitted early in the next unit
                if v_sb[prev[0]] is None:
                    drain_to(f"v{prev[0]}")
                norms_for(*prev)

            # ==== orchestration ====
            # prologue (direct): the first half of pair-0's qk projection
            # (enough for J=0 attention) and v for key blocks 0-7.
            # DMA issue order puts prologue-critical transfers first.
            qk0_closures, pstate0 = emit_qk_pair(0)
            # [dma_w, alloc_folds] + 4 groups (n0/n1 x 2 chunks) + fold0
            n_pro = 2 + 4 * (4 * qk_terms + 1) + 1
            qk0_closures[0]()          # wqk DMAs + stage alloc
            x8_chunk(0)
            nc.sync.dma_start(out=wva, in_=wva_d[:, :, :])
            nc.sync.dma_start(out=wvb, in_=wvb_d[:, :, :])
            x8_chunk(1)
            for c in qk0_closures[1:n_pro]:
                c()
            for tb in range(8):
                for c in v_group_closures(tb):
                    c()

            # fill queue: rest of qk0 (J1 needs it), v tb8-15, wout, pair-1 qk
            fill_q.append(lambda: x8_chunk(2))
            fill_q.append(lambda: x8_chunk(3))
            fill_q.extend(qk0_closures[n_pro:])
            fill_q.append("qk0full")
            for tb in range(8, 16):
                fill_q.extend(v_group_closures(tb))
                fill_q.append(f"v{tb}")
            for s in range(4):
                fill_q.append(lambda s=s: nc.sync.dma_start(out=wout_sb[s], in_=wout_d[s]))
            qk1_closures, pstate1 = emit_qk_pair(1)
            fill_q.extend(qk1_closures)
            fill_q.append("qk1")
            pair_states = [pstate0, pstate1, None, None]

            for pair in range(4):
                if pair > 0:
                    drain_to(f"qk{pair}")
                if 1 <= pair < 3:
                    cl, st = emit_qk_pair(pair + 1)
                    pair_states[pair + 1] = st
                    fill_q.extend(cl)
                    fill_q.append(f"qk{pair + 1}")
                ps = pair_states[pair]
                for J in range(2):
                    if pair == 0 and J == 1:
                        drain_to("qk0full")
                    for h in range(2):
                        lh = 2 * pair + h
                        pull = 6 if J == 1 else (4 if pair == 0 else 1)
                        post_cb = None
                        if pair == 3 and J == 1 and h == 1:
                            # half-A's closing mul (yT write) is emitted at
                            # block 13's pend flush; inject after it
                            def inject_out_a():
                                for tb in range(8, 12):
                                    fill_q.extend(emit_out_tb(tb))
                            post_cb = {14: inject_out_a}
                        emit_unit(ps, J, h, lh, pull, post_cb)
                        if pair == 3 and J == 0 and h == 1:
                            # out-projection for queries [0:1024]
                            while gpend:
                                gpend.pop(0)()
                            for tb in range(8):
                                fill_q.extend(emit_out_tb(tb))

            # flush deferred norms, drain leftovers, final out-projection
            while gpend:
                gpend.pop(0)()
            while fill_q:
                item = fill_q.popleft()
                if not isinstance(item, str):
                    item()
            for tb in range(12, 16):
                for c in emit_out_tb(tb):
                    c()

    nc.compile()
    return nc


def _q8(a):
    import ml_dtypes
    return np.ascontiguousarray(a.astype(ml_dtypes.float8_e4m3))


def _fold(a):
    """[E, C] -> [128, 8, C] with (p, 2g+j) = row 256g + 128j + p."""
    Edim, C = a.shape
    return np.ascontiguousarray(
        a.reshape(4, 2, 128, C).transpose(2, 0, 1, 3).reshape(128, 8, C)
    )


def _prep_in_maps(x, input_ids, w_qkv, w_out):
    import ml_dtypes

    in_maps = []
    for c in range(8):
        b, g = divmod(c, 2)
        hbase = 8 * g
        xT = np.ascontiguousarray(np.asarray(x[b], dtype=np.float32).T)  # [E, T]
        x8a = xT.astype(ml_dtypes.float8_e4m3).astype(np.float32)
        x8b = xT - x8a
        # q/k cols for the 4 pairs: [qA qB kA kB] each 64 -> [E, 256] per pair
        wqka = np.empty((4, 128, 8, 256), ml_dtypes.float8_e4m3)
        wqkb = np.empty((4, 128, 8, 256), ml_dtypes.float8_e4m3)
        for p in range(4):
            ha = hbase + 2 * p
            wc = np.concatenate(
                [
                    w_qkv[:, 64 * ha : 64 * ha + 128],
                    w_qkv[:, E + 64 * ha : E + 64 * ha + 128],
                ],
                axis=1,
            ) * SC
            wa = wc.astype(ml_dtypes.float8_e4m3).astype(np.float32)
            wqka[p] = _fold(wa).astype(ml_dtypes.float8_e4m3)
            wqkb[p] = _fold(wc - wa).astype(ml_dtypes.float8_e4m3)
        wv = np.asarray(
            w_qkv[:, 2 * E + 64 * hbase : 2 * E + 64 * (hbase + 8)], np.float32
        ) * SC
        wva = wv.astype(ml_dtypes.float8_e4m3).astype(np.float32)
        wvaf = _fold(wva).astype(ml_dtypes.float8_e4m3)
        wvbf = _fold(wv - wva).astype(ml_dtypes.float8_e4m3)
        wout = np.ascontiguousarray(
            np.asarray(w_out[512 * g : 512 * g + 512, :], np.float32).reshape(
                4, 128, E
            ).astype(ml_dtypes.bfloat16)
        )
        km = np.where(np.asarray(input_ids[b]) != 0, 0.0, NEG).astype(np.float32)
        km = np.ascontiguousarray(km.reshape(NKB, 128).T)
        in_maps.append({
            "x8a": _q8(_fold(x8a)),
            "x8b": _q8(_fold(x8b)),
            "wqka": np.ascontiguousarray(wqka),
            "wqkb": np.ascontiguousarray(wqkb),
            "wva": wvaf,
            "wvb": wvbf,
            "wout": wout,
            "km": km,
        })
    return in_maps


def kernel(x, input_ids, w_qkv, w_out, b_out, _trace=False):
    from concourse import bass_utils

    x = np.asarray(x, dtype=np.float32)
    w_qkv = np.asarray(w_qkv, dtype=np.float32)
    w_out = np.asarray(w_out, dtype=np.float32)
    b_out = np.asarray(b_out, dtype=np.float32)

    key = (QK_TERMS, V_TERMS)
    if key not in _cache:
        _cache[key] = _build_nc(QK_TERMS, V_TERMS)
    nc = _cache[key]

    in_maps = _prep_in_maps(x, input_ids, w_qkv, w_out)
    res = bass_utils.run_bass_kernel_spmd(
        nc, in_maps, core_ids=list(range(8)), trace=_trace,
    )
    out = np.empty((B, T, E), np.float32)
    for b in range(B):
        out[b] = res.results[2 * b]["out"] + res.results[2 * b + 1]["out"] + b_out
    if _trace:
        kernel.last_result = res
    return out



# revision 20
# speedup vs baseline: 1.3352x; 1.0390x over previous
"""Trainium2 Bass kernel for multi-head causal self-attention.

Reference computation (B=4, T=2048, E=1024, H=16 heads, D=64):
    qkv = x @ w_qkv;  q,k,v split
    scores = q @ k^T / sqrt(D),  causal + key-pad mask (input_ids==0)
    y = softmax(scores) @ v;  out = y @ w_out + b_out

Sharding over 8 cores: core c -> batch b = c//2, head-group g = c%2
(8 heads each).  Each core computes its heads' attention output and the
partial out-projection (contraction over its 512 y-dims); the host sums
the two partials per batch (w_out row-split tensor parallelism).

Per-core kernel strategy (cost-model-driven):
  - All projections run as fp8e4m3 DoubleRow matmuls (0.5 cycles/row,
    256-deep contraction per instruction = 4x fp32r throughput).
    Host-side residual decomposition (x = X1+X2, w = W1+W2 at fp8,
    computing X1W1+X1W2+X2W1) recovers ~bf16 accuracy.  Weights are
    pre-scaled by SC=32 so the residual terms stay out of fp8's
    subnormal range; the inverse scale is folded into the exp scale
    (scores) and the softmax-normalization constant (v path) for free.
  - Scores also run as fp8 DoubleRow: q,k are quantized to fp8 on the
    DVE during the psum->sbuf copy, and an SBUF->SBUF DMA folds the
    64 D-dims into [32 partitions, 2 k-tiles] DoubleRow layout.
  - Scores are computed transposed, sT [keys, q], so softmax(p) @ v is
    a matmul contracting over keys with v in natural [T, D] layout; a
    ones-column appended to v yields the denominator for free.
  - Exp (scalar engine) processes a [128, 1024] 2-psum-bank span per
    key block (both query halves in one instruction; the key-pad mask
    is a per-partition bias so one call per key block is legal).
    Softmax skips max-subtraction (scores are O(3); exp arg bounded).
  - Causal masking: off-diagonal blocks skipped; diagonal 128x128
    blocks exp'd unmasked, then the lower triangle of p is zeroed with
    a gpsimd affine_select (keeps DVE/Act off the critical path).
  - Software pipelining: QKV-projection matmuls for the next head pair
    (and the deferred V/out-projection work) are interleaved into the
    attention block loop through a fill queue, so the PE stays busy
    during the Act-bound attention phase.
"""

import numpy as np

B, T, E, H, D = 4, 2048, 1024, 16, 64
NEG = -1e30
SC = 32.0          # weight pre-scale (keeps fp8 residuals out of subnormals)
NKB = T // 128     # 16 key blocks
QK_TERMS = 2       # fp8 residual terms for the q/k projection (1, 2, or 3)
V_TERMS = 3        # fp8 residual terms for the v projection

_cache = {}


def _build_nc(qk_terms=QK_TERMS, v_terms=V_TERMS):
    import collections
    import concourse.mybir as mybir
    import concourse.tile as tile
    from concourse import bacc

    f32 = mybir.dt.float32
    bf16 = mybir.dt.bfloat16
    fp8 = mybir.dt.float8e4
    DR = mybir.MatmulPerfMode.DoubleRow
    Exp = mybir.ActivationFunctionType.Exp
    EXP_SCALE = 0.125 / (SC * SC)

    nc = bacc.Bacc("TRN2", target_bir_lowering=False)
    x8a_d = nc.dram_tensor("x8a", [128, 8, T], fp8, kind="ExternalInput")
    x8b_d = nc.dram_tensor("x8b", [128, 8, T], fp8, kind="ExternalInput")
    wqka_d = nc.dram_tensor("wqka", [4, 128, 8, 256], fp8, kind="ExternalInput")
    wqkb_d = nc.dram_tensor("wqkb", [4, 128, 8, 256], fp8, kind="ExternalInput")
    wva_d = nc.dram_tensor("wva", [128, 8, 512], fp8, kind="ExternalInput")
    wvb_d = nc.dram_tensor("wvb", [128, 8, 512], fp8, kind="ExternalInput")
    wout_d = nc.dram_tensor("wout", [4, 128, E], bf16, kind="ExternalInput")
    km_d = nc.dram_tensor("km", [128, NKB], f32, kind="ExternalInput")
    out_d = nc.dram_tensor("out", [T, E], f32, kind="ExternalOutput")

    with tile.TileContext(nc) as tc:
        with (
            tc.tile_pool(name="const", bufs=1) as cpool,
            tc.tile_pool(name="xp", bufs=1) as xpool,
            tc.tile_pool(name="wvp", bufs=1) as wvpool,
            tc.tile_pool(name="wqkp", bufs=2) as wqkpool,
            tc.tile_pool(name="wop", bufs=1) as wopool,
            tc.tile_pool(name="vp", bufs=1) as vpool,
            tc.tile_pool(name="stg", bufs=2) as stgpool,
            tc.tile_pool(name="qkf", bufs=2) as qkfpool,
            tc.tile_pool(name="ptp", bufs=6) as ptpool,
            tc.tile_pool(name="rp", bufs=2) as rpool,
            tc.tile_pool(name="ysbp", bufs=2) as ysbpool,
            tc.tile_pool(name="ytp", bufs=1) as ytpool,
            tc.tile_pool(name="otp", bufs=3) as otpool,
            tc.tile_pool(name="psA", bufs=2, space="PSUM") as psA,
            tc.tile_pool(name="psY", bufs=2, space="PSUM") as psY,
            tc.tile_pool(name="psM", bufs=2, space="PSUM") as psM,
        ):
            # ---- constants & resident inputs ----
            km_sb = cpool.tile([128, NKB], f32, tag="km", name="km_sb")
            nc.sync.dma_start(out=km_sb, in_=km_d[:, :])
            ones_inv = cpool.tile([65, 64], bf16, tag="oi", name="ones_inv")
            nc.vector.memset(ones_inv, 1.0 / SC)

            # x8 DMA'd in T-chunks; order: chunk0+wv first (v-proj needs them),
            # chunks 1-3 after, wout (needed only at out-proj) last.
            x8a = xpool.tile([128, 8, T], fp8, tag="xa", name="x8a")
            x8b = xpool.tile([128, 8, T], fp8, tag="xb", name="x8b")
            wva = wvpool.tile([128, 8, 512], fp8, tag="wva", name="wva")
            wvb = wvpool.tile([128, 8, 512], fp8, tag="wvb", name="wvb")

            def x8_chunk(n):
                sl = slice(512 * n, 512 * n + 512)
                nc.sync.dma_start(out=x8a[:, :, sl], in_=x8a_d[:, :, sl])
                nc.sync.dma_start(out=x8b[:, :, sl], in_=x8b_d[:, :, sl])

            wout_sb = []
            for s in range(4):
                t_ = wopool.tile([128, E], bf16, tag=f"wo{s}", name=f"wo{s}")
                wout_sb.append(t_)

            yT_all = [
                ytpool.tile([128, T], bf16, tag=f"yt{s}", name=f"yt{s}")
                for s in range(4)
            ]
            v_sb = [None] * NKB

            # x-side residual term list for a projection with given #terms:
            # (x_tile, w_is_resid) pairs
            def _terms(n):
                if n == 1:
                    return [(x8a, 0)]
                if n == 2:
                    return [(x8a, 0), (x8a, 1)]
                return [(x8a, 0), (x8a, 1), (x8b, 0)]

            # ---- emission helpers (closure lists, executed in program order) ----
            def v_group_closures(tb):
                # one [128 t, 512 vcols] psum group: v' = SC * x @ w_v
                st = {}

                def init():
                    vt = vpool.tile([128, 8, 65], bf16, tag=f"v{tb}", name=f"v{tb}")
                    # col 64 survives the copy; value SC makes den = SC*sum(p)
                    # so 1/den absorbs the weight pre-scale for free
                    nc.gpsimd.memset(vt, SC)
                    st["vt"] = vt
                    st["vp"] = psM.tile([128, 512], f32, tag="proj", name="vps")
                    mms = []
                    for xt, wr in _terms(v_terms):
                        for g in range(4):
                            mms.append((xt, (wvb if wr else wva), g))
                    st["mms"] = mms

                def mm_i(mi):
                    xt, wt, g = st["mms"][mi]
                    nc.tensor.matmul(
                        st["vp"],
                        lhsT=xt[:, 2 * g : 2 * g + 2, 128 * tb : 128 * tb + 128],
                        rhs=wt[:, 2 * g : 2 * g + 2, :],
                        start=(mi == 0), stop=(mi == len(st["mms"]) - 1),
                        perf_mode=DR,
                    )

                def first():
                    init()
                    mm_i(0)

                cls = [first]
                for mi in range(1, 4 * v_terms):
                    cls.append(lambda mi=mi: mm_i(mi))

                def copy():
                    nc.vector.tensor_copy(
                        out=st["vt"][:, :, 0:64],
                        in_=st["vp"].rearrange("p (h c) -> p h c", c=64),
                    )
                    v_sb[tb] = st["vt"]

                cls.append(copy)
                return cls

            def emit_qk_pair(pair):
                """qk-projection for one head pair -> folded fp8 q8f/k8f.
                Returns list of closures (executed via the fill queue)."""
                state = {}

                def dma_w():
                    wa = wqkpool.tile([128, 8, 256], fp8, tag="wa", name="wqka_sb")
                    nc.sync.dma_start(out=wa, in_=wqka_d[pair])
                    state["wa"] = wa
                    if qk_terms > 1:
                        wb = wqkpool.tile([128, 8, 256], fp8, tag="wb", name="wqkb_sb")
                        nc.sync.dma_start(out=wb, in_=wqkb_d[pair])
                        state["wb"] = wb
                    state["sq"] = stgpool.tile([128, T], fp8, tag="sq", name="stage_q")
                    state["sk"] = stgpool.tile([128, T], fp8, tag="sk", name="stage_k")

                closures = [dma_w]

                def mk_group(chunk, n):
                    # psum group: 128 qk-cols (chunk 0=q, 1=k), T-slice n
                    def go():
                        qp = psM.tile([128, 512], f32, tag="proj", name="qkps")
                        state["qp"] = qp
                        mms = []
                        for xt, wr in _terms(qk_terms):
                            for g in range(4):
                                mms.append((xt, state["wb"] if wr else state["wa"], g))
                        state["mms"] = mms

                        def mm_i(mi):
                            xt, wt, g = state["mms"][mi]
                            nc.tensor.matmul(
                                state["qp"],
                                lhsT=wt[:, 2 * g : 2 * g + 2,
                                        128 * chunk : 128 * chunk + 128],
                                rhs=xt[:, 2 * g : 2 * g + 2, 512 * n : 512 * n + 512],
                                start=(mi == 0), stop=(mi == len(state["mms"]) - 1),
                                perf_mode=DR,
                            )
                        return mm_i
                    # one closure creating the group+first mm, then the rest
                    def first():
                        state["mm_i"] = go()
                        state["mm_i"](0)
                    res = [first]
                    for mi in range(1, 4 * qk_terms):
                        res.append(lambda mi=mi: state["mm_i"](mi))
                    def copy():
                        stage = state["sq"] if chunk == 0 else state["sk"]
                        nc.vector.tensor_copy(
                            out=stage[:, 512 * n : 512 * n + 512], in_=state["qp"]
                        )
                    res.append(copy)
                    return res

                def alloc_folds():
                    state["qf"] = qkfpool.tile([64, 2, T], fp8, tag="qf", name="q8f")
                    state["kf"] = qkfpool.tile([64, 2, T], fp8, tag="kf", name="k8f")

                def folds(thalf):
                    tsl = slice(1024 * thalf, 1024 * thalf + 1024)
                    for dst, src in ((state["qf"], state["sq"]), (state["kf"], state["sk"])):
                        for h in range(2):
                            for j in range(2):
                                nc.sync.dma_start(
                                    out=dst[32 * h : 32 * h + 32, j, tsl],
                                    in_=src[64 * h + 32 * j : 64 * h + 32 * j + 32, tsl],
                                )

                closures.append(alloc_folds)
                # T-slices n=0,1 for both chunks first, then fold half 0,
                # then n=2,3 and fold half 1
                for n in (0, 1):
                    for chunk in range(2):
                        closures.extend(mk_group(chunk, n))
                closures.append(lambda: folds(0))
                for n in (2, 3):
                    for chunk in range(2):
                        closures.extend(mk_group(chunk, n))
                closures.append(lambda: folds(1))
                return closures, state

            def emit_out_tb(tb):
                ot = otpool.tile([128, E], f32, tag="ot", name="ot")
                cls = []
                for n2 in range(2):
                    def first(n2=n2):
                        op = psM.tile([128, 512], f32, tag="proj", name="ops")
                        emit_out_tb.op = op
                        nc.tensor.matmul(
                            op,
                            lhsT=yT_all[0][:, 128 * tb : 128 * tb + 128],
                            rhs=wout_sb[0][:, 512 * n2 : 512 * n2 + 512],
                            start=True, stop=False,
                        )
                    cls.append(first)
                    for s in range(1, 4):
                        def mid(s=s, n2=n2):
                            nc.tensor.matmul(
                                emit_out_tb.op,
                                lhsT=yT_all[s][:, 128 * tb : 128 * tb + 128],
                                rhs=wout_sb[s][:, 512 * n2 : 512 * n2 + 512],
                                start=False, stop=(s == 3),
                            )
                        cls.append(mid)
                    def copy(n2=n2):
                        nc.vector.tensor_copy(
                            out=ot[:, 512 * n2 : 512 * n2 + 512], in_=emit_out_tb.op
                        )
                    cls.append(copy)
                def dma():
                    nc.sync.dma_start(
                        out=out_d[128 * tb : 128 * tb + 128, :], in_=ot
                    )
                cls.append(dma)
                return cls

            # ---- fill queue machinery ----
            fill_q = collections.deque()
            markers_seen = set()

            def fill(n):
                done = 0
                while done < n and fill_q:
                    item = fill_q.popleft()
                    if isinstance(item, str):
                        markers_seen.add(item)
                        continue
                    item()
                    done += 1

            def drain_to(marker):
                while marker not in markers_seen:
                    item = fill_q.popleft()
                    if isinstance(item, str):
                        markers_seen.add(item)
                        continue
                    item()

            # ---- attention unit ----
            gpend = []             # deferred norm closures, cross-unit

            def emit_unit(pair_state, J, h, lh, pull, post_cb=None):
                qf, kf = pair_state["qf"], pair_state["kf"]
                kT = kf[32 * h : 32 * h + 32]
                qT = qf[32 * h : 32 * h + 32]
                nblk = 8 * J + 8
                y_half = [None, None]
                pend = gpend       # deferred closures (norm part2)
                prev = None        # (i, c0, pt_tile)

                def norms_for(i, c0, pt_t):
                    # emit pv for block, plus norm scheduling
                    for q2 in (0, 1):
                        lo = 512 * q2
                        ql = max(c0, lo)
                        if ql >= lo + 512:
                            continue
                        nlast = 8 * J + 3 if q2 == 0 else nblk - 1
                        if i == 0:
                            y_half[q2] = psY.tile([65, 512], f32, tag="y", name="yh")
                        nc.tensor.matmul(
                            y_half[q2][:, ql - lo : 512],
                            lhsT=v_sb[i][:, lh, :],
                            rhs=pt_t[:, ql : lo + 512],
                            start=(i == 0), stop=(i == nlast),
                        )
                        if i == nlast:
                            yh = y_half[q2]
                            # r = 1/(SC*den) at partition 0, then gpsimd
                            # broadcast to 64 partitions (SBUF, so the final
                            # mul has only one PSUM input)
                            rt = rpool.tile([65, 512], bf16, tag="r", name="rt")
                            with nc.allow_low_precision(reason="1/den in bf16; y is bf16 anyway"):
                                nc.vector.reciprocal(
                                    out=rt[0:1, :], in_=yh[64:65, :]
                                )
                            bc_sb = ysbpool.tile([64, 512], bf16, tag="ysb", name="bc_sb")
                            nc.gpsimd.partition_broadcast(bc_sb, rt[0:1, :])
                            def part2(q2=q2, yh=yh, bc_sb=bc_sb):
                                nc.vector.tensor_mul(
                                    yT_all[lh // 2][
                                        64 * (lh % 2) : 64 * (lh % 2) + 64,
                                        1024 * J + 512 * q2 : 1024 * J + 512 * q2 + 512,
                                    ],
                                    yh[0:64, :],
                                    bc_sb,
                                )
                            pend.append(part2)

                s2s = [None] * nblk

                def scores(i):
                    c0 = max(0, 128 * i - 1024 * J)
                    s2 = psA.tile([128, 1024], f32, tag="s", name="s2")
                    s2s[i] = s2
                    kblk = kT[:, :, 128 * i : 128 * i + 128]
                    if c0 < 512:
                        nc.tensor.matmul(
                            s2[:, c0:512],
                            lhsT=kblk,
                            rhs=qT[:, :, 1024 * J + c0 : 1024 * J + 512],
                            start=True, stop=True, perf_mode=DR,
                        )
                        nc.tensor.matmul(
                            s2[:, 512:1024],
                            lhsT=kblk,
                            rhs=qT[:, :, 1024 * J + 512 : 1024 * J + 1024],
                            start=True, stop=True, perf_mode=DR,
                        )
                    else:
                        nc.tensor.matmul(
                            s2[:, c0:1024],
                            lhsT=kblk,
                            rhs=qT[:, :, 1024 * J + c0 : 1024 * J + 1024],
                            start=True, stop=True, perf_mode=DR,
                        )

                scores(0)
                for i in range(nblk):
                    c0 = max(0, 128 * i - 1024 * J)
                    pt_t = ptpool.tile([128, 1024], bf16, tag="pt", name="pt")
                    nc.scalar.activation(
                        out=pt_t[:, c0:1024],
                        in_=s2s[i][:, c0:1024],
                        func=Exp,
                        bias=km_sb[:, i : i + 1],
                        scale=EXP_SCALE,
                    )
                    # next block's scores go ahead of PV/fill matmuls so the
                    # Act engine never waits on them
                    if i + 1 < nblk:
                        scores(i + 1)
                    if 128 * i >= 1024 * J:
                        nc.gpsimd.affine_select(
                            out=pt_t[:, c0 : c0 + 128],
                            in_=pt_t[:, c0 : c0 + 128],
                            compare_op=mybir.AluOpType.is_ge,
                            fill=0.0, base=0,
                            pattern=[[1, 128]], channel_multiplier=-1,
                        )
                    while pend:
                        pend.pop(0)()
                    if post_cb is not None and i in post_cb:
                        post_cb[i]()
                    fill(pull)
                    if prev is not None:
                        # hard ordering guarantee: v tile for block `prev`
                        # must be emitted before its pv matmul
                        if v_sb[prev[0]] is None:
                            drain_to(f"v{prev[0]}")
                        norms_for(*prev)
                    prev = (i, c0, pt_t)
                # final block's pv+norms; norm part2 closures stay in gpend
                # and are emitted early in the next unit
                if v_sb[prev[0]] is None:
                    drain_to(f"v{prev[0]}")
                norms_for(*prev)

            # ==== orchestration ====
            # prologue (direct): the first half of pair-0's qk projection
            # (enough for J=0 attention) and v for key blocks 0-7.
            # DMA issue order puts prologue-critical transfers first.
            qk0_closures, pstate0 = emit_qk_pair(0)
            # [dma_w, alloc_folds] + 4 groups (n0/n1 x 2 chunks) + fold0
            n_pro = 2 + 4 * (4 * qk_terms + 1) + 1
            qk0_closures[0]()          # wqk DMAs + stage alloc
            x8_chunk(0)
            nc.sync.dma_start(out=wva, in_=wva_d[:, :, :])
            nc.sync.dma_start(out=wvb, in_=wvb_d[:, :, :])
            x8_chunk(1)
            for c in qk0_closures[1:n_pro]:
                c()
            for tb in range(8):
                for c in v_group_closures(tb):
                    c()

            # fill queue: rest of qk0 (J1 needs it), v tb8-15, wout, pair-1 qk
            fill_q.append(lambda: x8_chunk(2))
            fill_q.append(lambda: x8_chunk(3))
            fill_q.extend(qk0_closures[n_pro:])
            fill_q.append("qk0full")
            for tb in range(8, 16):
                fill_q.extend(v_group_closures(tb))
                fill_q.append(f"v{tb}")
            for s in range(4):
                fill_q.append(lambda s=s: nc.sync.dma_start(out=wout_sb[s], in_=wout_d[s]))
            qk1_closures, pstate1 = emit_qk_pair(1)
            fill_q.extend(qk1_closures)
            fill_q.append("qk1")
            pair_states = [pstate0, pstate1, None, None]

            for pair in range(4):
                if pair > 0:
                    drain_to(f"qk{pair}")
                if 1 <= pair < 3:
                    cl, st = emit_qk_pair(pair + 1)
                    pair_states[pair + 1] = st
                    fill_q.extend(cl)
                    fill_q.append(f"qk{pair + 1}")
                ps = pair_states[pair]
                for J in range(2):
                    if pair == 0 and J == 1:
                        drain_to("qk0full")
                    for h in range(2):
                        lh = 2 * pair + h
                        pull = 6 if J == 1 else (4 if pair == 0 else 1)
                        post_cb = None
                        if pair == 3 and J == 1 and h == 1:
                            # half-A's closing mul (yT write) is emitted at
                            # block 13's pend flush; inject after it
                            def inject_out_a():
                                for tb in range(8, 12):
                                    fill_q.extend(emit_out_tb(tb))
                            post_cb = {14: inject_out_a}
                        emit_unit(ps, J, h, lh, pull, post_cb)
                        if pair == 3 and J == 0 and h == 1:
                            # out-projection for queries [0:1024]
                            while gpend:
                                gpend.pop(0)()
                            for tb in range(8):
                                fill_q.extend(emit_out_tb(tb))

            # flush deferred norms, drain leftovers, final out-projection
            while gpend:
                gpend.pop(0)()
            while fill_q:
                item = fill_q.popleft()
                if not isinstance(item, str):
                    item()
            for tb in range(12, 16):
                for c in emit_out_tb(tb):
                    c()

    nc.compile()
    return nc


def _q8(a):
    import ml_dtypes
    return np.ascontiguousarray(a.astype(ml_dtypes.float8_e4m3))


def _fold(a):
    """[E, C] -> [128, 8, C] with (p, 2g+j) = row 256g + 128j + p."""
    Edim, C = a.shape
    return np.ascontiguousarray(
        a.reshape(4, 2, 128, C).transpose(2, 0, 1, 3).reshape(128, 8, C)
    )


def _prep_in_maps(x, input_ids, w_qkv, w_out):
    import ml_dtypes

    in_maps = []
    for c in range(8):
        b, g = divmod(c, 2)
        hbase = 8 * g
        xT = np.ascontiguousarray(np.asarray(x[b], dtype=np.float32).T)  # [E, T]
        x8a = xT.astype(ml_dtypes.float8_e4m3).astype(np.float32)
        x8b = xT - x8a
        # q/k cols for the 4 pairs: [qA qB kA kB] each 64 -> [E, 256] per pair
        wqka = np.empty((4, 128, 8, 256), ml_dtypes.float8_e4m3)
        wqkb = np.empty((4, 128, 8, 256), ml_dtypes.float8_e4m3)
        for p in range(4):
            ha = hbase + 2 * p
            wc = np.concatenate(
                [
                    w_qkv[:, 64 * ha : 64 * ha + 128],
                    w_qkv[:, E + 64 * ha : E + 64 * ha + 128],
                ],
                axis=1,
            ) * SC
            wa = wc.astype(ml_dtypes.float8_e4m3).astype(np.float32)
            wqka[p] = _fold(wa).astype(ml_dtypes.float8_e4m3)
            wqkb[p] = _fold(wc - wa).astype(ml_dtypes.float8_e4m3)
        wv = np.asarray(
            w_qkv[:, 2 * E + 64 * hbase : 2 * E + 64 * (hbase + 8)], np.float32
        ) * SC
        wva = wv.astype(ml_dtypes.float8_e4m3).astype(np.float32)
        wvaf = _fold(wva).astype(ml_dtypes.float8_e4m3)
        wvbf = _fold(wv - wva).astype(ml_dtypes.float8_e4m3)
        wout = np.ascontiguousarray(
            np.asarray(w_out[512 * g : 512 * g + 512, :], np.float32).reshape(
                4, 128, E
            ).astype(ml_dtypes.bfloat16)
        )
        km = np.where(np.asarray(input_ids[b]) != 0, 0.0, NEG).astype(np.float32)
        km = np.ascontiguousarray(km.reshape(NKB, 128).T)
        in_maps.append({
            "x8a": _q8(_fold(x8a)),
            "x8b": _q8(_fold(x8b)),
            "wqka": np.ascontiguousarray(wqka),
            "wqkb": np.ascontiguousarray(wqkb),
            "wva": wvaf,
            "wvb": wvbf,
            "wout": wout,
            "km": km,
        })
    return in_maps


def kernel(x, input_ids, w_qkv, w_out, b_out, _trace=False):
    from concourse import bass_utils

    x = np.asarray(x, dtype=np.float32)
    w_qkv = np.asarray(w_qkv, dtype=np.float32)
    w_out = np.asarray(w_out, dtype=np.float32)
    b_out = np.asarray(b_out, dtype=np.float32)

    key = (QK_TERMS, V_TERMS)
    if key not in _cache:
        _cache[key] = _build_nc(QK_TERMS, V_TERMS)
    nc = _cache[key]

    in_maps = _prep_in_maps(x, input_ids, w_qkv, w_out)
    res = bass_utils.run_bass_kernel_spmd(
        nc, in_maps, core_ids=list(range(8)), trace=_trace,
    )
    out = np.empty((B, T, E), np.float32)
    for b in range(B):
        out[b] = res.results[2 * b]["out"] + res.results[2 * b + 1]["out"] + b_out
    if _trace:
        kernel.last_result = res
    return out



# revision 21
# speedup vs baseline: 1.3610x; 1.0193x over previous
"""Trainium2 Bass kernel for multi-head causal self-attention.

Reference computation (B=4, T=2048, E=1024, H=16 heads, D=64):
    qkv = x @ w_qkv;  q,k,v split
    scores = q @ k^T / sqrt(D),  causal + key-pad mask (input_ids==0)
    y = softmax(scores) @ v;  out = y @ w_out + b_out

Sharding over 8 cores: core c -> batch b = c//2, head-group g = c%2
(8 heads each).  Each core computes its heads' attention output and the
partial out-projection (contraction over its 512 y-dims); the host sums
the two partials per batch (w_out row-split tensor parallelism).

Per-core kernel strategy (cost-model-driven):
  - All projections run as fp8e4m3 DoubleRow matmuls (0.5 cycles/row,
    256-deep contraction per instruction = 4x fp32r throughput).
    Host-side residual decomposition (x = X1+X2, w = W1+W2 at fp8,
    computing X1W1+X1W2+X2W1) recovers ~bf16 accuracy.  Weights are
    pre-scaled by SC=32 so the residual terms stay out of fp8's
    subnormal range; the inverse scale is folded into the exp scale
    (scores) and the softmax-normalization constant (v path) for free.
  - Scores also run as fp8 DoubleRow: q,k are quantized to fp8 on the
    DVE during the psum->sbuf copy, and an SBUF->SBUF DMA folds the
    64 D-dims into [32 partitions, 2 k-tiles] DoubleRow layout.
  - Scores are computed transposed, sT [keys, q], so softmax(p) @ v is
    a matmul contracting over keys with v in natural [T, D] layout; a
    ones-column appended to v yields the denominator for free.
  - Exp (scalar engine) processes a [128, 1024] 2-psum-bank span per
    key block (both query halves in one instruction; the key-pad mask
    is a per-partition bias so one call per key block is legal).
    Softmax skips max-subtraction (scores are O(3); exp arg bounded).
  - Causal masking: off-diagonal blocks skipped; diagonal 128x128
    blocks exp'd unmasked, then the lower triangle of p is zeroed with
    a gpsimd affine_select (keeps DVE/Act off the critical path).
  - Software pipelining: QKV-projection matmuls for the next head pair
    (and the deferred V/out-projection work) are interleaved into the
    attention block loop through a fill queue, so the PE stays busy
    during the Act-bound attention phase.
"""

import numpy as np

B, T, E, H, D = 4, 2048, 1024, 16, 64
NEG = -1e30
SC = 32.0          # weight pre-scale (keeps fp8 residuals out of subnormals)
NKB = T // 128     # 16 key blocks
QK_TERMS = 2       # fp8 residual terms for the q/k projection (1, 2, or 3)
V_TERMS = 3        # fp8 residual terms for the v projection

_cache = {}


def _build_nc(qk_terms=QK_TERMS, v_terms=V_TERMS):
    import collections
    import concourse.mybir as mybir
    import concourse.tile as tile
    from concourse import bacc

    f32 = mybir.dt.float32
    bf16 = mybir.dt.bfloat16
    fp8 = mybir.dt.float8e4
    DR = mybir.MatmulPerfMode.DoubleRow
    Exp = mybir.ActivationFunctionType.Exp
    EXP_SCALE = 0.125 / (SC * SC)

    nc = bacc.Bacc("TRN2", target_bir_lowering=False)
    x8a_d = nc.dram_tensor("x8a", [128, 8, T], fp8, kind="ExternalInput")
    x8b_d = nc.dram_tensor("x8b", [128, 8, T], fp8, kind="ExternalInput")
    wqka_d = nc.dram_tensor("wqka", [4, 128, 8, 256], fp8, kind="ExternalInput")
    wqkb_d = nc.dram_tensor("wqkb", [4, 128, 8, 256], fp8, kind="ExternalInput")
    wva_d = nc.dram_tensor("wva", [128, 8, 512], fp8, kind="ExternalInput")
    wvb_d = nc.dram_tensor("wvb", [128, 8, 512], fp8, kind="ExternalInput")
    wout_d = nc.dram_tensor("wout", [4, 128, E], bf16, kind="ExternalInput")
    km_d = nc.dram_tensor("km", [128, NKB], f32, kind="ExternalInput")
    out_d = nc.dram_tensor("out", [T, E], f32, kind="ExternalOutput")

    with tile.TileContext(nc) as tc:
        with (
            tc.tile_pool(name="const", bufs=1) as cpool,
            tc.tile_pool(name="xp", bufs=1) as xpool,
            tc.tile_pool(name="wvp", bufs=1) as wvpool,
            tc.tile_pool(name="wqkp", bufs=2) as wqkpool,
            tc.tile_pool(name="wop", bufs=1) as wopool,
            tc.tile_pool(name="vp", bufs=1) as vpool,
            tc.tile_pool(name="stg", bufs=2) as stgpool,
            tc.tile_pool(name="qkf", bufs=2) as qkfpool,
            tc.tile_pool(name="ptp", bufs=6) as ptpool,
            tc.tile_pool(name="rp", bufs=2) as rpool,
            tc.tile_pool(name="ysbp", bufs=2) as ysbpool,
            tc.tile_pool(name="ytp", bufs=1) as ytpool,
            tc.tile_pool(name="otp", bufs=3) as otpool,
            tc.tile_pool(name="psA", bufs=2, space="PSUM") as psA,
            tc.tile_pool(name="psY", bufs=2, space="PSUM") as psY,
            tc.tile_pool(name="psM", bufs=2, space="PSUM") as psM,
        ):
            # ---- constants & resident inputs ----
            km_sb = cpool.tile([128, NKB], f32, tag="km", name="km_sb")
            nc.sync.dma_start(out=km_sb, in_=km_d[:, :])
            ones_inv = cpool.tile([65, 64], bf16, tag="oi", name="ones_inv")
            nc.vector.memset(ones_inv, 1.0 / SC)

            # x8 DMA'd in T-chunks; order: chunk0+wv first (v-proj needs them),
            # chunks 1-3 after, wout (needed only at out-proj) last.
            x8a = xpool.tile([128, 8, T], fp8, tag="xa", name="x8a")
            x8b = xpool.tile([128, 8, T], fp8, tag="xb", name="x8b")
            wva = wvpool.tile([128, 8, 512], fp8, tag="wva", name="wva")
            wvb = wvpool.tile([128, 8, 512], fp8, tag="wvb", name="wvb")

            def x8_chunk(n):
                sl = slice(512 * n, 512 * n + 512)
                nc.sync.dma_start(out=x8a[:, :, sl], in_=x8a_d[:, :, sl])
                nc.sync.dma_start(out=x8b[:, :, sl], in_=x8b_d[:, :, sl])

            wout_sb = []
            for s in range(4):
                t_ = wopool.tile([128, E], bf16, tag=f"wo{s}", name=f"wo{s}")
                wout_sb.append(t_)

            yT_all = [
                ytpool.tile([128, T], bf16, tag=f"yt{s}", name=f"yt{s}")
                for s in range(4)
            ]
            v_sb = [None] * NKB

            # x-side residual term list for a projection with given #terms:
            # (x_tile, w_is_resid) pairs
            def _terms(n):
                if n == 1:
                    return [(x8a, 0)]
                if n == 2:
                    return [(x8a, 0), (x8a, 1)]
                return [(x8a, 0), (x8a, 1), (x8b, 0)]

            # ---- emission helpers (closure lists, executed in program order) ----
            def v_group_closures(tb):
                # one [128 t, 512 vcols] psum group: v' = SC * x @ w_v
                st = {}

                def init():
                    vt = vpool.tile([128, 8, 65], bf16, tag=f"v{tb}", name=f"v{tb}")
                    # col 64 survives the copy; value SC makes den = SC*sum(p)
                    # so 1/den absorbs the weight pre-scale for free
                    nc.gpsimd.memset(vt, SC)
                    st["vt"] = vt
                    st["vp"] = psM.tile([128, 512], f32, tag="proj", name="vps")
                    mms = []
                    for xt, wr in _terms(v_terms):
                        for g in range(4):
                            mms.append((xt, (wvb if wr else wva), g))
                    st["mms"] = mms

                def mm_i(mi):
                    xt, wt, g = st["mms"][mi]
                    nc.tensor.matmul(
                        st["vp"],
                        lhsT=xt[:, 2 * g : 2 * g + 2, 128 * tb : 128 * tb + 128],
                        rhs=wt[:, 2 * g : 2 * g + 2, :],
                        start=(mi == 0), stop=(mi == len(st["mms"]) - 1),
                        perf_mode=DR,
                    )

                def first():
                    init()
                    mm_i(0)

                cls = [first]
                for mi in range(1, 4 * v_terms):
                    cls.append(lambda mi=mi: mm_i(mi))

                def copy():
                    nc.vector.tensor_copy(
                        out=st["vt"][:, :, 0:64],
                        in_=st["vp"].rearrange("p (h c) -> p h c", c=64),
                    )
                    v_sb[tb] = st["vt"]

                cls.append(copy)
                return cls

            def emit_qk_pair(pair):
                """qk-projection for one head pair -> folded fp8 q8f/k8f.
                Returns list of closures (executed via the fill queue)."""
                state = {}

                def dma_w():
                    wa = wqkpool.tile([128, 8, 256], fp8, tag="wa", name="wqka_sb")
                    nc.sync.dma_start(out=wa, in_=wqka_d[pair])
                    state["wa"] = wa
                    if qk_terms > 1:
                        wb = wqkpool.tile([128, 8, 256], fp8, tag="wb", name="wqkb_sb")
                        nc.sync.dma_start(out=wb, in_=wqkb_d[pair])
                        state["wb"] = wb
                    state["sq"] = stgpool.tile([128, T], fp8, tag="sq", name="stage_q")
                    state["sk"] = stgpool.tile([128, T], fp8, tag="sk", name="stage_k")

                closures = [dma_w]

                def mk_group(chunk, n):
                    # psum group: 128 qk-cols (chunk 0=q, 1=k), T-slice n
                    def go():
                        qp = psM.tile([128, 512], f32, tag="proj", name="qkps")
                        state["qp"] = qp
                        mms = []
                        for xt, wr in _terms(qk_terms):
                            for g in range(4):
                                mms.append((xt, state["wb"] if wr else state["wa"], g))
                        state["mms"] = mms

                        def mm_i(mi):
                            xt, wt, g = state["mms"][mi]
                            nc.tensor.matmul(
                                state["qp"],
                                lhsT=wt[:, 2 * g : 2 * g + 2,
                                        128 * chunk : 128 * chunk + 128],
                                rhs=xt[:, 2 * g : 2 * g + 2, 512 * n : 512 * n + 512],
                                start=(mi == 0), stop=(mi == len(state["mms"]) - 1),
                                perf_mode=DR,
                            )
                        return mm_i
                    # one closure creating the group+first mm, then the rest
                    def first():
                        state["mm_i"] = go()
                        state["mm_i"](0)
                    res = [first]
                    for mi in range(1, 4 * qk_terms):
                        res.append(lambda mi=mi: state["mm_i"](mi))
                    def copy():
                        stage = state["sq"] if chunk == 0 else state["sk"]
                        nc.vector.tensor_copy(
                            out=stage[:, 512 * n : 512 * n + 512], in_=state["qp"]
                        )
                    res.append(copy)
                    return res

                def alloc_folds():
                    state["qf"] = qkfpool.tile([64, 2, T], fp8, tag="qf", name="q8f")
                    state["kf"] = qkfpool.tile([64, 2, T], fp8, tag="kf", name="k8f")

                def folds(thalf):
                    tsl = slice(1024 * thalf, 1024 * thalf + 1024)
                    for dst, src in ((state["qf"], state["sq"]), (state["kf"], state["sk"])):
                        for h in range(2):
                            for j in range(2):
                                nc.sync.dma_start(
                                    out=dst[32 * h : 32 * h + 32, j, tsl],
                                    in_=src[64 * h + 32 * j : 64 * h + 32 * j + 32, tsl],
                                )

                closures.append(alloc_folds)
                # T-slices n=0,1 for both chunks first, then fold half 0,
                # then n=2,3 and fold half 1
                for n in (0, 1):
                    for chunk in range(2):
                        closures.extend(mk_group(chunk, n))
                closures.append(lambda: folds(0))
                for n in (2, 3):
                    for chunk in range(2):
                        closures.extend(mk_group(chunk, n))
                closures.append(lambda: folds(1))
                return closures, state

            def emit_out_tb(tb):
                ot = otpool.tile([128, E], f32, tag="ot", name="ot")
                st = {}
                cls = []
                for n2 in range(2):
                    def first(n2=n2):
                        op = psM.tile([128, 512], f32, tag="proj", name="ops")
                        st["op"] = op
                        nc.tensor.matmul(
                            op,
                            lhsT=yT_all[0][:, 128 * tb : 128 * tb + 128],
                            rhs=wout_sb[0][:, 512 * n2 : 512 * n2 + 512],
                            start=True, stop=False,
                        )
                    cls.append(first)
                    for s in range(1, 4):
                        def mid(s=s, n2=n2):
                            nc.tensor.matmul(
                                st["op"],
                                lhsT=yT_all[s][:, 128 * tb : 128 * tb + 128],
                                rhs=wout_sb[s][:, 512 * n2 : 512 * n2 + 512],
                                start=False, stop=(s == 3),
                            )
                        cls.append(mid)
                    def copy(n2=n2):
                        nc.vector.tensor_copy(
                            out=ot[:, 512 * n2 : 512 * n2 + 512], in_=st["op"]
                        )
                    cls.append(copy)
                def dma():
                    nc.sync.dma_start(
                        out=out_d[128 * tb : 128 * tb + 128, :], in_=ot
                    )
                cls.append(dma)
                return cls

            # ---- fill queue machinery ----
            fill_q = collections.deque()
            markers_seen = set()

            def fill(n):
                done = 0
                while done < n and fill_q:
                    item = fill_q.popleft()
                    if isinstance(item, str):
                        markers_seen.add(item)
                        continue
                    item()
                    done += 1

            def drain_to(marker):
                while marker not in markers_seen:
                    item = fill_q.popleft()
                    if isinstance(item, str):
                        markers_seen.add(item)
                        continue
                    item()

            # ---- attention unit ----
            gpend = []             # deferred norm closures, cross-unit

            def emit_unit(pair_state, J, h, lh, pull, post_cb=None):
                qf, kf = pair_state["qf"], pair_state["kf"]
                kT = kf[32 * h : 32 * h + 32]
                qT = qf[32 * h : 32 * h + 32]
                nblk = 8 * J + 8
                y_half = [None, None]
                pend = gpend       # deferred closures (norm part2)
                prev = None        # (i, c0, pt_tile)

                def norms_for(i, c0, pt_t):
                    # emit pv for block, plus norm scheduling
                    for q2 in (0, 1):
                        lo = 512 * q2
                        ql = max(c0, lo)
                        if ql >= lo + 512:
                            continue
                        nlast = 8 * J + 3 if q2 == 0 else nblk - 1
                        if i == 0:
                            y_half[q2] = psY.tile([65, 512], f32, tag="y", name="yh")
                        nc.tensor.matmul(
                            y_half[q2][:, ql - lo : 512],
                            lhsT=v_sb[i][:, lh, :],
                            rhs=pt_t[:, ql : lo + 512],
                            start=(i == 0), stop=(i == nlast),
                        )
                        if i == nlast:
                            yh = y_half[q2]
                            # r = 1/(SC*den) at partition 0, then gpsimd
                            # broadcast to 64 partitions (SBUF, so the final
                            # mul has only one PSUM input)
                            rt = rpool.tile([65, 512], bf16, tag="r", name="rt")
                            with nc.allow_low_precision(reason="1/den in bf16; y is bf16 anyway"):
                                nc.vector.reciprocal(
                                    out=rt[0:1, :], in_=yh[64:65, :]
                                )
                            bc_sb = ysbpool.tile([64, 512], bf16, tag="ysb", name="bc_sb")
                            nc.gpsimd.partition_broadcast(bc_sb, rt[0:1, :])
                            def part2(q2=q2, yh=yh, bc_sb=bc_sb):
                                nc.vector.tensor_mul(
                                    yT_all[lh // 2][
                                        64 * (lh % 2) : 64 * (lh % 2) + 64,
                                        1024 * J + 512 * q2 : 1024 * J + 512 * q2 + 512,
                                    ],
                                    yh[0:64, :],
                                    bc_sb,
                                )
                            pend.append(part2)

                s2s = [None] * nblk

                def scores(i):
                    c0 = max(0, 128 * i - 1024 * J)
                    s2 = psA.tile([128, 1024], f32, tag="s", name="s2")
                    s2s[i] = s2
                    kblk = kT[:, :, 128 * i : 128 * i + 128]
                    if c0 < 512:
                        nc.tensor.matmul(
                            s2[:, c0:512],
                            lhsT=kblk,
                            rhs=qT[:, :, 1024 * J + c0 : 1024 * J + 512],
                            start=True, stop=True, perf_mode=DR,
                        )
                        nc.tensor.matmul(
                            s2[:, 512:1024],
                            lhsT=kblk,
                            rhs=qT[:, :, 1024 * J + 512 : 1024 * J + 1024],
                            start=True, stop=True, perf_mode=DR,
                        )
                    else:
                        nc.tensor.matmul(
                            s2[:, c0:1024],
                            lhsT=kblk,
                            rhs=qT[:, :, 1024 * J + c0 : 1024 * J + 1024],
                            start=True, stop=True, perf_mode=DR,
                        )

                scores(0)
                for i in range(nblk):
                    c0 = max(0, 128 * i - 1024 * J)
                    pt_t = ptpool.tile([128, 1024], bf16, tag="pt", name="pt")
                    nc.scalar.activation(
                        out=pt_t[:, c0:1024],
                        in_=s2s[i][:, c0:1024],
                        func=Exp,
                        bias=km_sb[:, i : i + 1],
                        scale=EXP_SCALE,
                    )
                    # next block's scores go ahead of PV/fill matmuls so the
                    # Act engine never waits on them
                    if i + 1 < nblk:
                        scores(i + 1)
                    if 128 * i >= 1024 * J:
                        nc.gpsimd.affine_select(
                            out=pt_t[:, c0 : c0 + 128],
                            in_=pt_t[:, c0 : c0 + 128],
                            compare_op=mybir.AluOpType.is_ge,
                            fill=0.0, base=0,
                            pattern=[[1, 128]], channel_multiplier=-1,
                        )
                    while pend:
                        pend.pop(0)()
                    if post_cb is not None and i in post_cb:
                        post_cb[i]()
                    fill(pull)
                    if prev is not None:
                        # hard ordering guarantee: v tile for block `prev`
                        # must be emitted before its pv matmul
                        if v_sb[prev[0]] is None:
                            drain_to(f"v{prev[0]}")
                        norms_for(*prev)
                    prev = (i, c0, pt_t)
                # final block's pv+norms; norm part2 closures stay in gpend
                # and are emitted early in the next unit
                if v_sb[prev[0]] is None:
                    drain_to(f"v{prev[0]}")
                norms_for(*prev)

            # ==== orchestration ====
            # prologue (direct): the first half of pair-0's qk projection
            # (enough for J=0 attention) and v for key blocks 0-7.
            # DMA issue order puts prologue-critical transfers first.
            qk0_closures, pstate0 = emit_qk_pair(0)
            # [dma_w, alloc_folds] + 4 groups (n0/n1 x 2 chunks) + fold0
            n_pro = 2 + 4 * (4 * qk_terms + 1) + 1
            qk0_closures[0]()          # wqk DMAs + stage alloc
            x8_chunk(0)
            nc.sync.dma_start(out=wva, in_=wva_d[:, :, :])
            nc.sync.dma_start(out=wvb, in_=wvb_d[:, :, :])
            x8_chunk(1)
            for c in qk0_closures[1:n_pro]:
                c()
            for c in v_group_closures(0):
                c()

            # fill queue: v1-7 first (drained on demand by J0 chains; their
            # operands are resident so forced drains never stall), then the
            # rest of qk0 (J1 needs it), v tb8-15, wout, pair-1 qk
            for tb in range(1, 8):
                fill_q.extend(v_group_closures(tb))
                fill_q.append(f"v{tb}")
            fill_q.append(lambda: x8_chunk(2))
            fill_q.append(lambda: x8_chunk(3))
            fill_q.extend(qk0_closures[n_pro:])
            fill_q.append("qk0full")
            for tb in range(8, 16):
                fill_q.extend(v_group_closures(tb))
                fill_q.append(f"v{tb}")
            for s in range(4):
                fill_q.append(lambda s=s: nc.sync.dma_start(out=wout_sb[s], in_=wout_d[s]))
            qk1_closures, pstate1 = emit_qk_pair(1)
            fill_q.extend(qk1_closures)
            fill_q.append("qk1")
            pair_states = [pstate0, pstate1, None, None]

            for pair in range(4):
                if pair > 0:
                    drain_to(f"qk{pair}")
                if 1 <= pair < 3:
                    cl, st = emit_qk_pair(pair + 1)
                    pair_states[pair + 1] = st
                    fill_q.extend(cl)
                    fill_q.append(f"qk{pair + 1}")
                ps = pair_states[pair]
                for J in range(2):
                    if pair == 0 and J == 1:
                        drain_to("qk0full")
                    for h in range(2):
                        lh = 2 * pair + h
                        pull = 6 if J == 1 else (4 if pair == 0 else 1)
                        post_cb = None
                        if pair == 3 and J == 1 and h == 1:
                            # yT for qb is complete at the pend flush of block
                            # qb+2; stream its out-projection directly
                            def mk_out(tb):
                                def go():
                                    for c in emit_out_tb(tb):
                                        c()
                                return go
                            post_cb = {11: mk_out(8), 12: mk_out(9),
                                       13: mk_out(10), 14: mk_out(11),
                                       15: mk_out(12)}
                        emit_unit(ps, J, h, lh, pull, post_cb)
                        if pair == 3 and J == 0 and h == 1:
                            # out-projection for queries [0:1024]
                            while gpend:
                                gpend.pop(0)()
                            for tb in range(8):
                                fill_q.extend(emit_out_tb(tb))

            # flush deferred norms, drain leftovers, final out-projection
            while gpend:
                gpend.pop(0)()
            while fill_q:
                item = fill_q.popleft()
                if not isinstance(item, str):
                    item()
            for tb in range(13, 16):
                for c in emit_out_tb(tb):
                    c()

    nc.compile()
    return nc


def _q8(a):
    import ml_dtypes
    return np.ascontiguousarray(a.astype(ml_dtypes.float8_e4m3))


def _fold(a):
    """[E, C] -> [128, 8, C] with (p, 2g+j) = row 256g + 128j + p."""
    Edim, C = a.shape
    return np.ascontiguousarray(
        a.reshape(4, 2, 128, C).transpose(2, 0, 1, 3).reshape(128, 8, C)
    )


def _prep_in_maps(x, input_ids, w_qkv, w_out):
    import ml_dtypes

    in_maps = []
    for c in range(8):
        b, g = divmod(c, 2)
        hbase = 8 * g
        xT = np.ascontiguousarray(np.asarray(x[b], dtype=np.float32).T)  # [E, T]
        x8a = xT.astype(ml_dtypes.float8_e4m3).astype(np.float32)
        x8b = xT - x8a
        # q/k cols for the 4 pairs: [qA qB kA kB] each 64 -> [E, 256] per pair
        wqka = np.empty((4, 128, 8, 256), ml_dtypes.float8_e4m3)
        wqkb = np.empty((4, 128, 8, 256), ml_dtypes.float8_e4m3)
        for p in range(4):
            ha = hbase + 2 * p
            wc = np.concatenate(
                [
                    w_qkv[:, 64 * ha : 64 * ha + 128],
                    w_qkv[:, E + 64 * ha : E + 64 * ha + 128],
                ],
                axis=1,
            ) * SC
            wa = wc.astype(ml_dtypes.float8_e4m3).astype(np.float32)
            wqka[p] = _fold(wa).astype(ml_dtypes.float8_e4m3)
            wqkb[p] = _fold(wc - wa).astype(ml_dtypes.float8_e4m3)
        wv = np.asarray(
            w_qkv[:, 2 * E + 64 * hbase : 2 * E + 64 * (hbase + 8)], np.float32
        ) * SC
        wva = wv.astype(ml_dtypes.float8_e4m3).astype(np.float32)
        wvaf = _fold(wva).astype(ml_dtypes.float8_e4m3)
        wvbf = _fold(wv - wva).astype(ml_dtypes.float8_e4m3)
        wout = np.ascontiguousarray(
            np.asarray(w_out[512 * g : 512 * g + 512, :], np.float32).reshape(
                4, 128, E
            ).astype(ml_dtypes.bfloat16)
        )
        km = np.where(np.asarray(input_ids[b]) != 0, 0.0, NEG).astype(np.float32)
        km = np.ascontiguousarray(km.reshape(NKB, 128).T)
        in_maps.append({
            "x8a": _q8(_fold(x8a)),
            "x8b": _q8(_fold(x8b)),
            "wqka": np.ascontiguousarray(wqka),
            "wqkb": np.ascontiguousarray(wqkb),
            "wva": wvaf,
            "wvb": wvbf,
            "wout": wout,
            "km": km,
        })
    return in_maps


def kernel(x, input_ids, w_qkv, w_out, b_out, _trace=False):
    from concourse import bass_utils

    x = np.asarray(x, dtype=np.float32)
    w_qkv = np.asarray(w_qkv, dtype=np.float32)
    w_out = np.asarray(w_out, dtype=np.float32)
    b_out = np.asarray(b_out, dtype=np.float32)

    key = (QK_TERMS, V_TERMS)
    if key not in _cache:
        _cache[key] = _build_nc(QK_TERMS, V_TERMS)
    nc = _cache[key]

    in_maps = _prep_in_maps(x, input_ids, w_qkv, w_out)
    res = bass_utils.run_bass_kernel_spmd(
        nc, in_maps, core_ids=list(range(8)), trace=_trace,
    )
    out = np.empty((B, T, E), np.float32)
    for b in range(B):
        out[b] = res.results[2 * b]["out"] + res.results[2 * b + 1]["out"] + b_out
    if _trace:
        kernel.last_result = res
    return out

